# revision 1
# baseline (speedup 1.0000x reference)
"""Bidirectional cross-attention Trainium2 kernel.

Sharding: (batch, head) units. B=2, H=12 -> 24 units over 8 cores:
core c handles batch b = c // 4 and heads 3*(c%4) .. 3*(c%4)+2.
Each core computes the full attention for its 3 heads plus the partial
output projections; the host sums the per-core partial projections
(the "all-reduce after the output projections"), transposes back, adds
biases and concatenates the two branches.

Schedule (v2):
- DMA order: (qkw_i, vw_i, xt_i) interleaved, then (qksw_i, st_i), vsw,
  projections. PE starts on x-side projections at ~4us while the src
  side streams.
- Passes are software-pipelined by 2 iterations (sim/exp for iter i+2
  emitted before the acc matmuls of iter i) so the PE never waits on
  the ACT exp at pass/half starts.
- Each pass's normalization is split into three positioned fillers
  (psc / bb / mul) woven into the NEXT pass's iteration stream.
- W2 output-projection matmuls are zero-padded to K=128 (K=64 matmuls
  stream at half rate on TRN2).
- vas (src-side v-projection) chunks stream INTO pass p2u0's early
  iterations: chunk i is only needed at iteration i.
"""

import os
import sys
from contextlib import ExitStack

import numpy as np

sys.path.insert(0, "/opt/trn_rl_repo")

import ml_dtypes  # noqa: E402

import concourse.bass as bass  # noqa: E402
import concourse.tile as tile  # noqa: E402
from concourse import bacc, mybir  # noqa: E402
from concourse import bass_utils  # noqa: E402

# ---------------------------------------------------------------- constants
P = 128          # partitions
C = 768          # channels
CB = C // P      # 6 channel blocks
NH = 3           # heads per core
D2 = 128         # qk dims per head (2*HEAD_DIM)
DH = 64          # v dims per head
QW = NH * D2     # 384
VW = NH * DH     # 192
H = 12
B = 2
N_CORES = 8
SCALE = DH ** -0.5

BF = mybir.dt.bfloat16
F32 = mybir.dt.float32

_PROG_CACHE: dict[int, "bacc.Bacc"] = {}


def _build_program(NT: int) -> "bacc.Bacc":
    """Build+schedule+compile the per-core Bass program (SPMD: same program
    on all 8 cores, per-core data differs)."""
    NCH = NT // P      # 128-row chunks
    N5 = NT // 512     # 512-col chunks
    N10 = NT // 1024   # 1024-col chunks

    nc = bacc.Bacc(
        "TRN2",
        target_bir_lowering=False,
        debug=False,
        num_devices=N_CORES,
    )

    xT_d = nc.dram_tensor("xT", [C, NT], BF, kind="ExternalInput").ap()
    srcT_d = nc.dram_tensor("srcT", [C, NT], BF, kind="ExternalInput").ap()
    qkw_d = nc.dram_tensor("qk_wT", [C, QW], BF, kind="ExternalInput").ap()
    qksw_d = nc.dram_tensor("qks_wT", [C, QW], BF, kind="ExternalInput").ap()
    vw_d = nc.dram_tensor("v_wT", [C, VW], BF, kind="ExternalInput").ap()
    vsw_d = nc.dram_tensor("vs_wT", [C, VW], BF, kind="ExternalInput").ap()
    pjw_d = nc.dram_tensor("projT", [VW, C], BF, kind="ExternalInput").ap()
    pjsw_d = nc.dram_tensor("projsT", [VW, C], BF, kind="ExternalInput").ap()
    ident_d = nc.dram_tensor("ident", [P, P], BF, kind="ExternalInput").ap()
    oy_d = nc.dram_tensor("out_y", [C, NT], BF, kind="ExternalOutput").ap()
    oys_d = nc.dram_tensor("out_ys", [C, NT], BF, kind="ExternalOutput").ap()

    with tile.TileContext(nc) as tc, ExitStack() as ctx:
        sb = ctx.enter_context(tc.tile_pool(name="sb", bufs=1, space="SBUF"))
        ps = ctx.enter_context(tc.tile_pool(name="ps", bufs=2, space="PSUM"))

        # ---------------- constants / zero-fills (no DRAM deps)
        ones64 = sb.tile([P, DH], BF, tag="ones64")
        nc.gpsimd.memset(ones64[:], 1.0)
        zb = sb.tile([P, 1], F32, tag="zb")
        nc.gpsimd.memset(zb[:], 0.0)
        ident = sb.tile([P, P], BF, tag="ident")
        nc.sync.dma_start(ident[:], ident_d[:])

        # v tensors, ones-augmented: vax[:, u*NCH*65 + 65*i + e] with
        # e in [0,64] = v values, e == 64 = 1.0 (softmax denominator row).
        vax = sb.tile([P, NH * NCH * 65], BF, tag="vax", name="vax")
        vas = sb.tile([P, NH * NCH * 65], BF, tag="vas", name="vas")
        vaxr = vax.rearrange("p (u i e) -> p u i e", u=NH, e=65)
        vasr = vas.rearrange("p (u i e) -> p u i e", u=NH, e=65)
        nc.gpsimd.memset(vaxr[:, :, :, DH:65], 1.0)
        nc.gpsimd.memset(vasr[:, :, :, DH:65], 1.0)

        # head stacks for the output projections. Y2/YS2 are padded to 128
        # partitions (rows DH:P zero) so the W2 oproj matmul runs at K=128.
        Y01 = sb.tile([P, NT], BF, tag="Y01")
        Y2 = sb.tile([P, NT], BF, tag="Y2")
        YS01 = sb.tile([P, NT], BF, tag="YS01")
        YS2 = sb.tile([P, NT], BF, tag="YS2")
        nc.gpsimd.memset(Y2[DH:P, :], 0.0)
        nc.gpsimd.memset(YS2[DH:P, :], 0.0)

        # ---------------- input loads (emission order == DMA priority)
        xt, qkw, vw = [], [], []
        for i in range(CB):
            qt = sb.tile([P, QW], BF, tag="qkw", bufs=CB, name=f"qkw{i}")
            nc.sync.dma_start(qt[:], qkw_d[P * i:P * (i + 1), :])
            qkw.append(qt)
            vt = sb.tile([P, VW], BF, tag="vw", bufs=CB, name=f"vw{i}")
            nc.sync.dma_start(vt[:], vw_d[P * i:P * (i + 1), :])
            vw.append(vt)
            t = sb.tile([P, NT], BF, tag="xt", bufs=CB, name=f"xt{i}")
            nc.sync.dma_start(t[:], xT_d[P * i:P * (i + 1), :])
            xt.append(t)
        st, qksw = [], []
        for i in range(CB):
            qt = sb.tile([P, QW], BF, tag="qksw", bufs=CB, name=f"qksw{i}")
            nc.sync.dma_start(qt[:], qksw_d[P * i:P * (i + 1), :])
            qksw.append(qt)
            t = sb.tile([P, NT], BF, tag="st", bufs=CB, name=f"st{i}")
            nc.sync.dma_start(t[:], srcT_d[P * i:P * (i + 1), :])
            st.append(t)
        vsw = []
        for i in range(CB):
            vt = sb.tile([P, VW], BF, tag="vsw", bufs=CB, name=f"vsw{i}")
            nc.sync.dma_start(vt[:], vsw_d[P * i:P * (i + 1), :])
            vsw.append(vt)
        # output projection weights; W2 halves padded to K=128 with zeros.
        pjs1 = sb.tile([P, C], BF, tag="pjs1")
        nc.sync.dma_start(pjs1[:], pjsw_d[0:P, :])
        pjs2 = sb.tile([P, C], BF, tag="pjs2")
        nc.sync.dma_start(pjs2[0:DH, :], pjsw_d[P:VW, :])
        nc.gpsimd.memset(pjs2[DH:P, :], 0.0)
        pj1 = sb.tile([P, C], BF, tag="pj1")
        nc.sync.dma_start(pj1[:], pjw_d[0:P, :])
        pj2 = sb.tile([P, C], BF, tag="pj2")
        nc.sync.dma_start(pj2[0:DH, :], pjw_d[P:VW, :])
        nc.gpsimd.memset(pj2[DH:P, :], 0.0)

        # ---------------- per-head transposed QK projections
        # qkt[u][d2, n] = sum_c qk_wT[c, 128u+d2] * xT[c, n]
        def gen_qkt_head(act_tiles, w_tiles, tag, u, split_copy=False):
            t = sb.tile([P, NT], BF, tag=tag, bufs=NH, name=f"{tag}{u}")
            for jj in range(N10):
                pst = ps.tile([P, 1024], F32, tag="ps_sim",
                              name=f"ps_{tag}{u}_{jj}")
                for h2 in range(2):
                    lo = 1024 * jj + 512 * h2
                    for cb in range(CB):
                        nc.tensor.matmul(
                            pst[:, 512 * h2:512 * (h2 + 1)],
                            lhsT=w_tiles[cb][:, D2 * u:D2 * (u + 1)],
                            rhs=act_tiles[cb][:, lo:lo + 512],
                            start=(cb == 0), stop=(cb == CB - 1),
                        )
                if split_copy:
                    # pass-1-start critical path: halve latency, ACT idle
                    nc.scalar.copy(
                        t[:, 1024 * jj:1024 * jj + 512], pst[:, 0:512])
                    nc.vector.tensor_copy(
                        t[:, 1024 * jj + 512:1024 * (jj + 1)],
                        pst[:, 512:1024])
                else:
                    nc.vector.tensor_copy(
                        t[:, 1024 * jj:1024 * (jj + 1)], pst[:])
            return t

        # filler variant: one 512-col psum group per closure.
        def qkt_fillers(act_tiles, w_tiles, tag, u):
            t = sb.tile([P, NT], BF, tag=tag, bufs=NH, name=f"{tag}{u}")

            def mk(q):
                def f():
                    pst = ps.tile([P, 512], F32, tag="ps_small",
                                  name=f"psq_{tag}{u}_{q}")
                    for cb in range(CB):
                        nc.tensor.matmul(
                            pst[:],
                            lhsT=w_tiles[cb][:, D2 * u:D2 * (u + 1)],
                            rhs=act_tiles[cb][:, 512 * q:512 * (q + 1)],
                            start=(cb == 0), stop=(cb == CB - 1),
                        )
                    nc.vector.tensor_copy(t[:, 512 * q:512 * (q + 1)], pst[:])
                return f
            return t, [mk(q) for q in range(N5)]

        # v in natural layout into the pre-built augmented tiles.
        def gen_vaug(act_tiles, w_tiles, vr, tag, chunks):
            for i in chunks:
                psv = ps.tile([P, VW], F32, tag="ps_small",
                              name=f"psv_{tag}{i}")
                for cb in range(CB):
                    nc.tensor.matmul(
                        psv[:],
                        lhsT=act_tiles[cb][:, P * i:P * (i + 1)],
                        rhs=w_tiles[cb][:],
                        start=(cb == 0), stop=(cb == CB - 1),
                    )
                nc.vector.tensor_copy(
                    vr[:, :, i, 0:DH],
                    psv.rearrange("p (u e) -> p u e", e=DH),
                )

        # ---------------- phase A: x-side first (DMA streams src side)
        qkt = [None] * NH
        qkst = [None] * NH
        qkt[0] = gen_qkt_head(xt, qkw, "qkt", 0)
        gen_vaug(xt, vw, vaxr, "vax", range(0, 12))
        qkt[1] = gen_qkt_head(xt, qkw, "qkt", 1)
        gen_vaug(xt, vw, vaxr, "vax", range(12, NCH))
        qkst[0] = gen_qkt_head(st, qksw, "qkst", 0, split_copy=True)

        def stack_slice(s01, s2, u):
            if u == 0:
                return s01[0:DH, :]
            if u == 1:
                return s01[DH:P, :]
            return s2[0:DH, :]

        # ---------------- one attention pass (one softmax direction)
        # Software-pipelined by 2 iterations: the acc matmuls for iter i are
        # emitted after sim+exp of iter i+2, so the PE FIFO never stalls on
        # the ACT exp latency. `fillers` maps iteration index -> closures.
        def attention_pass(u, qa, qb, va, pname, fillers=None, tail=False):
            fillers = dict(fillers or {})
            accS = sb.tile([65, NT], BF, tag="accS", bufs=2,
                           name=f"accS_{pname}")
            pend = []

            def drain(n):
                while len(pend) > n:
                    pend.pop(0)()

            it = 0
            for half in range(N10):
                acc = ps.tile([65, 1024], F32, tag="ps_acc", bufs=1,
                              name=f"acc_{pname}_{half}")
                for i in range(NCH):
                    est = sb.tile([P, 1024], BF, tag="es", bufs=8,
                                  name=f"es_{pname}_{half}_{i}")
                    pst = ps.tile([P, 1024], F32, tag="ps_sim",
                                  name=f"ps_{pname}_{half}_{i}")
                    for h2 in range(2):
                        lo = 1024 * half + 512 * h2
                        nc.tensor.matmul(
                            pst[:, 512 * h2:512 * (h2 + 1)],
                            lhsT=qa[:, P * i:P * (i + 1)],
                            rhs=qb[:, lo:lo + 512],
                            start=True, stop=True,
                        )
                    nc.scalar.activation(
                        est[:], pst[:],
                        mybir.ActivationFunctionType.Exp, bias=zb[:],
                    )
                    drain(2)

                    def mk_acc(acc=acc, est=est, i=i):
                        def f():
                            vsl = va[:, u * NCH * 65 + 65 * i:
                                     u * NCH * 65 + 65 * (i + 1)]
                            for k in range(2):
                                nc.tensor.matmul(
                                    acc[:, 512 * k:512 * (k + 1)],
                                    lhsT=vsl,
                                    rhs=est[:, 512 * k:512 * (k + 1)],
                                    start=(i == 0), stop=(i == NCH - 1),
                                )
                        return f
                    pend.append(mk_acc())
                    for f in fillers.pop(it, []):
                        f()
                    it += 1

                def mk_copy(acc=acc, half=half):
                    def f():
                        if tail and half == N10 - 1:
                            # norm chain start is latency-critical: split
                            # the copy across ACT+DVE in parallel.
                            nc.scalar.copy(
                                accS[:, 1024 * half:1024 * half + 512],
                                acc[:, 0:512])
                            nc.vector.tensor_copy(
                                accS[:, 1024 * half + 512:1024 * (half + 1)],
                                acc[:, 512:1024])
                        else:
                            nc.vector.tensor_copy(
                                accS[:, 1024 * half:1024 * (half + 1)],
                                acc[:])
                    return f
                pend.append(mk_copy())
            drain(0)
            for fl in fillers.values():   # flush leftovers (shouldn't happen)
                for f in fl:
                    f()

            # normalization, split into three independently-positioned parts:
            #   psc: spread denominator row into psum columns (PE) + recip
            #        on [128, NCH] (DVE, all lanes) + cast
            #   bb : broadcast 1/denom back to [DH, NT] via identity matmuls
            #   mul: dst = accS[0:DH] * bb  (DVE)
            state = {}

            def norm_psc():
                psc = ps.tile([P, 512], F32, tag="ps_small",
                              name=f"psc_{pname}")
                for j in range(NCH):
                    nc.tensor.matmul(
                        psc[:, j:j + 1],
                        lhsT=accS[64:65, P * j:P * (j + 1)],
                        rhs=ones64[64:65, 0:1],
                        start=True, stop=True,
                    )
                rcpF = sb.tile([P, NCH], F32, tag="rcpF", bufs=2,
                               name=f"rcpF_{pname}")
                nc.vector.reciprocal(rcpF[:], psc[:, 0:NCH])
                rcpT = sb.tile([P, NCH], BF, tag="rcpT", bufs=2,
                               name=f"rcpT_{pname}")
                nc.vector.tensor_copy(rcpT[:], rcpF[:])
                state["rcpT"] = rcpT

            def norm_bb():
                rcpT = state["rcpT"]
                bb = sb.tile([DH, NT], BF, tag="bb", bufs=2,
                             name=f"bb_{pname}")
                for k in range(N5):
                    pso = ps.tile([DH, 512], F32, tag="ps_small",
                                  name=f"psbb_{pname}_{k}")
                    for jj in range(4):
                        j = 4 * k + jj
                        col = rcpT[:, j:j + 1]
                        lhsT_b = bass.AP(col.tensor, col.offset,
                                         [col.ap[0], [0, DH]])
                        nc.tensor.matmul(
                            pso[:, P * jj:P * (jj + 1)], lhsT=lhsT_b,
                            rhs=ident[:], start=True, stop=True,
                        )
                    if tail:  # ACT is free after the last pass: split copy
                        nc.scalar.copy(bb[:, 512 * k:512 * k + 256],
                                       pso[:, 0:256])
                        nc.vector.tensor_copy(
                            bb[:, 512 * k + 256:512 * (k + 1)],
                            pso[:, 256:512])
                    else:
                        nc.vector.tensor_copy(bb[:, 512 * k:512 * (k + 1)],
                                              pso[:])
                state["bb"] = bb

            def norm_mul(dst, ks=None):
                bb = state["bb"]
                if ks is None:
                    nc.vector.tensor_mul(dst, accS[0:DH, :], bb[:])
                else:
                    for k in ks:
                        sl = slice(512 * k, 512 * (k + 1))
                        nc.vector.tensor_mul(dst[:, sl], accS[0:DH, sl],
                                             bb[:, sl])
            return norm_psc, norm_bb, norm_mul

        # ---------------- partial output projection (both W halves K=128)
        def oproj_cc(S1, S2, W1, W2, out_d, pname, cc):
            for k in range(N5):
                pso = ps.tile([P, 512], F32, tag="ps_small",
                              name=f"pso_{pname}_{cc}_{k}")
                nc.tensor.matmul(
                    pso[:], lhsT=W1[:, P * cc:P * (cc + 1)],
                    rhs=S1[:, 512 * k:512 * (k + 1)],
                    start=True, stop=False,
                )
                nc.tensor.matmul(
                    pso[:], lhsT=W2[:, P * cc:P * (cc + 1)],
                    rhs=S2[:, 512 * k:512 * (k + 1)],
                    start=False, stop=True,
                )
                stg = sb.tile([P, 512], BF, tag="stg", bufs=6,
                              name=f"stg_{pname}_{cc}_{k}")
                nc.vector.tensor_copy(stg[:], pso[:])
                nc.sync.dma_start(
                    out_d[P * cc:P * (cc + 1), 512 * k:512 * (k + 1)],
                    stg[:],
                )

        # ---------------- filler construction
        qkt[2], fqkt2 = qkt_fillers(xt, qkw, "qkt", 2)
        qkst[1], fqkst1 = qkt_fillers(st, qksw, "qkst", 1)
        qkst[2], fqkst2 = qkt_fillers(st, qksw, "qkst", 2)
        fvas = [
            (lambda c0: lambda: gen_vaug(st, vsw, vasr, "vas",
                                         range(c0, c0 + 4)))(c0)
            for c0 in range(0, NCH, 4)
        ]
        fosrc = [
            (lambda cc: lambda: oproj_cc(YS01, YS2, pjs1, pjs2, oys_d,
                                         "osrc", cc))(cc)
            for cc in range(CB)
        ]

        # ---------------- main schedule
        # p1 passes (qa = x-side): dst = YS (src-branch outputs)
        # p2 passes (qa = src-side): dst = Y (x-branch outputs)
        # Filler placement rules: a filler must never sit inside the pass
        # that consumes its output (forward-order RAW is untracked), and
        # load should spread so each pass rides its ACT (exp) floor.
        norms = {}
        # fqkst1[0] emitted inline: its matmuls bridge the qkst0-copy wait
        # so the PE enters pass 1 busy and at full clock.
        fqkst1[0]()
        norms["p1u0"] = attention_pass(
            0, qkt[0], qkst[0], vax, "p1u0",
            fillers={5: [fqkst1[1]], 10: [fqkst1[2]], 15: [fqkst1[3]],
                     20: [fqkt2[0]], 23: [fqkt2[1]],
                     26: [fqkt2[2]], 29: [fqkt2[3]]})
        n = norms["p1u0"]
        norms["p1u1"] = attention_pass(
            1, qkt[1], qkst[1], vax, "p1u1",
            fillers={3: [n[0]], 6: [fqkst2[0]], 9: [n[1]],
                     11: [lambda: n[2](stack_slice(YS01, YS2, 0))],
                     13: [fqkst2[1]], 16: [fqkst2[2]], 19: [fqkst2[3]],
                     22: [fvas[0]], 26: [fvas[1]]})
        n = norms["p1u1"]
        norms["p1u2"] = attention_pass(
            2, qkt[2], qkst[2], vax, "p1u2",
            fillers={3: [n[0]], 7: [n[1]],
                     9: [lambda: n[2](stack_slice(YS01, YS2, 1))],
                     13: [fvas[2]], 19: [fvas[3]]})
        n = norms["p1u2"]
        norms["p2u0"] = attention_pass(
            0, qkst[0], qkt[0], vas, "p2u0",
            fillers={3: [n[0]], 7: [n[1]],
                     9: [lambda: n[2](stack_slice(YS01, YS2, 2))],
                     18: [fosrc[0]], 25: [fosrc[1]]})
        n = norms["p2u0"]
        norms["p2u1"] = attention_pass(
            1, qkst[1], qkt[1], vas, "p2u1",
            fillers={3: [n[0]], 7: [n[1]],
                     9: [lambda: n[2](stack_slice(Y01, Y2, 0))],
                     15: [fosrc[2]], 22: [fosrc[3]]})
        n = norms["p2u1"]
        # fosrc[4]/[5] sit at the very end of the last pass: they have no
        # on-chip consumers, and running right up to the pass boundary
        # carries a hot PE clock into the tail's norm chain.
        norms["p2u2"] = attention_pass(
            2, qkst[2], qkt[2], vas, "p2u2",
            fillers={3: [n[0]], 7: [n[1]],
                     9: [lambda: n[2](stack_slice(Y01, Y2, 1))],
                     24: [fosrc[4]], 29: [fosrc[5]]},
            tail=True)

        # ---------------- tail: last norm + x-branch output projection,
        # interleaved per 512-token chunk so the PE stays dense.
        npsc, nbb, nmul = norms["p2u2"]
        npsc()
        nbb()
        # all 4 norm muls first: the projection groups then see Y2 ready
        # after a single pipeline fill instead of stalling once per chunk.
        nmul(stack_slice(Y01, Y2, 2), ks=range(N5))
        # Pair projection groups into [P, 1024] ps_sim tiles (idle after the
        # last pass): 4 groups in flight keeps the PE stream dense.
        units = [(cc, k) for k in range(N5) for cc in range(CB)]
        for p in range(0, len(units), 2):
            pst = ps.tile([P, 1024], F32, tag="ps_sim", name=f"psoy_{p}")
            for q in range(2):
                cc, k = units[p + q]
                pso = pst[:, 512 * q:512 * (q + 1)]
                nc.tensor.matmul(
                    pso, lhsT=pj1[:, P * cc:P * (cc + 1)],
                    rhs=Y01[:, 512 * k:512 * (k + 1)],
                    start=True, stop=False,
                )
                nc.tensor.matmul(
                    pso, lhsT=pj2[:, P * cc:P * (cc + 1)],
                    rhs=Y2[:, 512 * k:512 * (k + 1)],
                    start=False, stop=True,
                )
            for q in range(2):
                cc, k = units[p + q]
                pso = pst[:, 512 * q:512 * (q + 1)]
                stg = sb.tile([P, 512], BF, tag="stg", bufs=6,
                              name=f"stgy_{cc}_{k}")
                # split each staging copy across ACT+DVE in parallel
                nc.scalar.copy(stg[:, 0:256], pso[:, 0:256])
                nc.vector.tensor_copy(stg[:, 256:512], pso[:, 256:512])
                nc.sync.dma_start(
                    oy_d[P * cc:P * (cc + 1), 512 * k:512 * (k + 1)],
                    stg[:],
                )

    nc.compile()
    return nc


def _get_program(NT: int) -> "bacc.Bacc":
    if NT not in _PROG_CACHE:
        _PROG_CACHE[NT] = _build_program(NT)
    return _PROG_CACHE[NT]


def make_in_maps(x, src, qk_w, qk_src_w, v_w, v_src_w, proj_w, proj_src_w):
    """Host-side sharding: per-core input dicts (pure data marshalling)."""
    bf = ml_dtypes.bfloat16

    def prep(a):
        return np.ascontiguousarray(a).astype(bf)

    in_maps = []
    for c in range(N_CORES):
        b = c // 4
        heads = [3 * (c % 4) + j for j in range(NH)]
        qk_rows = np.concatenate([qk_w[D2 * h:D2 * (h + 1), :] for h in heads])
        qks_rows = np.concatenate(
            [qk_src_w[D2 * h:D2 * (h + 1), :] for h in heads])
        v_rows = np.concatenate([v_w[DH * h:DH * (h + 1), :] for h in heads])
        vs_rows = np.concatenate(
            [v_src_w[DH * h:DH * (h + 1), :] for h in heads])
        pj_cols = np.concatenate(
            [proj_w[:, DH * h:DH * (h + 1)] for h in heads], axis=1)
        pjs_cols = np.concatenate(
            [proj_src_w[:, DH * h:DH * (h + 1)] for h in heads], axis=1)
        in_maps.append({
            "ident": np.eye(P).astype(ml_dtypes.bfloat16),
            "xT": prep(x[b].T),
            "srcT": prep(src[b].T),
            "qk_wT": prep(qk_rows.T * SCALE),
            "qks_wT": prep(qks_rows.T),
            "v_wT": prep(v_rows.T),
            "vs_wT": prep(vs_rows.T),
            "projT": prep(pj_cols.T),
            "projsT": prep(pjs_cols.T),
        })
    return in_maps


LAST_RESULTS = None  # BassKernelResults of the most recent kernel() call
_HOOK_DONE = False


def _install_ntff_hook():
    """The agent image's antenv lacks axon_hooks; inject a stub module and
    register the ctypes NTFF profile hook so trace=True yields exec times."""
    global _HOOK_DONE
    if _HOOK_DONE:
        return
    try:
        import types
        import antenv  # noqa: F401
        if "antenv.axon_hooks" not in sys.modules:
            mod = types.ModuleType("antenv.axon_hooks")
            _hook = [None]
            mod.set_axon_ntff_profile_hook = lambda h: _hook.__setitem__(0, h)
            mod.get_axon_ntff_profile_hook = lambda: _hook[0]
            sys.modules["antenv.axon_hooks"] = mod
        import trn_agent_boot.trn_boot as _tb
        from antenv.axon_hooks import set_axon_ntff_profile_hook
        set_axon_ntff_profile_hook(
            _tb._ntff_profile_via_ctypes("/opt/axon/libaxon_pjrt.so"))
        _HOOK_DONE = True
    except Exception as e:  # profiling is best-effort
        print(f"ntff hook install failed: {e}", file=sys.stderr)


def kernel(x, src, qk_w, qk_src_w, v_w, v_src_w, proj_w, proj_b,
           proj_src_w, proj_src_b):
    global LAST_RESULTS
    x = np.asarray(x, np.float32)
    src = np.asarray(src, np.float32)
    NT = x.shape[1]

    in_maps = make_in_maps(
        x, src,
        np.asarray(qk_w, np.float32), np.asarray(qk_src_w, np.float32),
        np.asarray(v_w, np.float32), np.asarray(v_src_w, np.float32),
        np.asarray(proj_w, np.float32), np.asarray(proj_src_w, np.float32),
    )

    nc = _get_program(NT)
    trace = bool(int(os.environ.get("BCA_TRACE", "0")))
    if trace:
        _install_ntff_hook()
    res = bass_utils.run_bass_kernel_spmd(
        nc, in_maps, core_ids=list(range(N_CORES)), trace=trace,
    )
    LAST_RESULTS = res

    # host gather: sum partial projections over the 4 cores of each batch,
    # transpose back, add biases, concat branches.
    oy = np.zeros((B, NT, C), np.float32)
    oys = np.zeros((B, NT, C), np.float32)
    for c in range(N_CORES):
        b = c // 4
        oy[b] += np.asarray(res.results[c]["out_y"], np.float32).T
        oys[b] += np.asarray(res.results[c]["out_ys"], np.float32).T
    oy += np.asarray(proj_b, np.float32)
    oys += np.asarray(proj_src_b, np.float32)
    return np.concatenate([oy, oys], axis=-1).astype(np.float32)



# revision 12
# speedup vs baseline: 1.0736x; 1.0736x over previous
"""Bidirectional cross-attention Trainium2 kernel (v3).

Sharding: (batch, head) units. B=2, H=12 -> 24 units over 8 cores:
core c handles batch b = c // 4 and heads 3*(c%4) .. 3*(c%4)+2.
Each core computes the full attention for its 3 heads plus the partial
output projections; the host sums the per-core partial projections,
transposes back, adds biases and concatenates the two branches.

v3 restructure vs v2: each head's two softmax directions share ONE sim
computation. exp(sim) tiles are transposed SBUF->SBUF by the DMA XBAR
(16x128 tiles; runs on otherwise-idle DMA engines) into a per-half
arena laid out [m-chunk(8), m(128), n(2048)]; the second direction's
accumulation consumes the arena directly. This removes the per-head
sim recompute (~33k PE cycles/head) and halves ACT exp work.

Schedule:
- One merged pass per head, two m-halves of 1024 cols each. Per
  iteration (n-chunk i): 2x sim matmul [128,512] -> exp -> est tile;
  one XBAR-transpose DMA per est tile into the arena; dir-1 acc
  matmuls pipelined 2 iterations behind (pend/drain).
- dir-2 acc (16 matmuls per (half, n-half)) is woven as fillers into
  the NEXT phase's iteration stream; halves are combined via a bf16
  SBUF partial (part2) + DVE add.
- PSUM: ps_sim 3x[128,512] + ps_acc1 [128,1024] + ps_acc2 [128,1024]
  + ps_small [128,512] = 8 banks.
- Y01/Y2/YS01/YS2 output stacks reuse the xt tag's SBUF slots (all
  x-side gen is emitted before the stack allocs), freeing room for
  the 2x4MB arena.
"""

import os
import sys
from contextlib import ExitStack

import numpy as np

sys.path.insert(0, "/opt/trn_rl_repo")

import ml_dtypes  # noqa: E402

import concourse.bass as bass  # noqa: E402
import concourse.tile as tile  # noqa: E402
from concourse import bacc, mybir  # noqa: E402
from concourse import bass_utils  # noqa: E402

# ---------------------------------------------------------------- constants
P = 128          # partitions
C = 768          # channels
CB = C // P      # 6 channel blocks
NH = 3           # heads per core
D2 = 128         # qk dims per head (2*HEAD_DIM)
DH = 64          # v dims per head
QW = NH * D2     # 384
VW = NH * DH     # 192
H = 12
B = 2
N_CORES = 8
SCALE = DH ** -0.5

BF = mybir.dt.bfloat16
F32 = mybir.dt.float32

_PROG_CACHE: dict[int, "bacc.Bacc"] = {}


def _build_program(NT: int) -> "bacc.Bacc":
    """Build+schedule+compile the per-core Bass program (SPMD: same program
    on all 8 cores, per-core data differs)."""
    NCH = NT // P      # 128-row chunks (16)
    N5 = NT // 512     # 512-col groups (4)
    NAR = NT // 256    # arena m-blocks per half (8)

    nc = bacc.Bacc(
        "TRN2",
        target_bir_lowering=False,
        debug=False,
        num_devices=N_CORES,
    )

    xT_d = nc.dram_tensor("xT", [C, NT], BF, kind="ExternalInput").ap()
    srcT_d = nc.dram_tensor("srcT", [C, NT], BF, kind="ExternalInput").ap()
    qkw_d = nc.dram_tensor("qk_wT", [C, QW], BF, kind="ExternalInput").ap()
    qksw_d = nc.dram_tensor("qks_wT", [C, QW], BF, kind="ExternalInput").ap()
    vw_d = nc.dram_tensor("v_wT", [C, VW], BF, kind="ExternalInput").ap()
    vsw_d = nc.dram_tensor("vs_wT", [C, VW], BF, kind="ExternalInput").ap()
    pjw_d = nc.dram_tensor("projT", [VW, C], BF, kind="ExternalInput").ap()
    pjsw_d = nc.dram_tensor("projsT", [VW, C], BF, kind="ExternalInput").ap()
    ident_d = nc.dram_tensor("ident", [P, P], BF, kind="ExternalInput").ap()
    oy_d = nc.dram_tensor("out_y", [C, NT], BF, kind="ExternalOutput").ap()
    oys_d = nc.dram_tensor("out_ys", [C, NT], BF, kind="ExternalOutput").ap()

    with tile.TileContext(nc) as tc, ExitStack() as ctx:
        sb = ctx.enter_context(tc.tile_pool(name="sb", bufs=1, space="SBUF"))
        ps = ctx.enter_context(tc.tile_pool(name="ps", bufs=1, space="PSUM"))

        def pt_sim(cols, name):
            return ps.tile([P, cols], F32, tag="ps_sim", bufs=3, name=name,
                           padded_shape=[P, 512])

        def pt_a1(cols, name, rows=P):
            return ps.tile([rows, cols], F32, tag="ps_acc1", bufs=1,
                           name=name, padded_shape=[P, 1024])

        def pt_a2(cols, name, rows=P):
            return ps.tile([rows, cols], F32, tag="ps_acc2", bufs=1,
                           name=name, padded_shape=[P, 1024])

        def pt_small(cols, name):
            return ps.tile([P, cols], F32, tag="ps_small", bufs=1, name=name,
                           padded_shape=[P, 512])

        # ---------------- constants / zero-fills (no DRAM deps)
        ones64 = sb.tile([P, DH], BF, tag="ones64")
        nc.gpsimd.memset(ones64[:], 1.0)
        zb = sb.tile([P, 1], F32, tag="zb")
        nc.gpsimd.memset(zb[:], 0.0)
        ident = sb.tile([P, P], BF, tag="ident")
        nc.sync.dma_start(ident[:], ident_d[:])

        # v tensors, ones-augmented: va[:, u*NCH*65 + 65*i + e] with
        # e in [0,64) = v values, e == 64 = 1.0 (softmax denominator row).
        vax = sb.tile([P, NH * NCH * 65], BF, tag="vax", name="vax")
        vas = sb.tile([P, NH * NCH * 65], BF, tag="vas", name="vas")
        vaxr = vax.rearrange("p (u i e) -> p u i e", u=NH, e=65)
        vasr = vas.rearrange("p (u i e) -> p u i e", u=NH, e=65)
        nc.gpsimd.memset(vaxr[:, :, :, DH:65], 1.0)
        nc.gpsimd.memset(vasr[:, :, :, DH:65], 1.0)

        # ---------------- input loads (emission order == DMA priority)
        xt, qkw, vw = [], [], []
        for i in range(CB):
            qt = sb.tile([P, QW], BF, tag="qkw", bufs=CB, name=f"qkw{i}")
            nc.sync.dma_start(qt[:], qkw_d[P * i:P * (i + 1), :])
            qkw.append(qt)
            vt = sb.tile([P, VW], BF, tag="vw", bufs=CB, name=f"vw{i}")
            nc.sync.dma_start(vt[:], vw_d[P * i:P * (i + 1), :])
            vw.append(vt)
            t = sb.tile([P, NT], BF, tag="xt", bufs=CB, name=f"xt{i}")
            nc.sync.dma_start(t[:], xT_d[P * i:P * (i + 1), :])
            xt.append(t)
        st, qksw = [], []
        for i in range(CB):
            qt = sb.tile([P, QW], BF, tag="qksw", bufs=CB, name=f"qksw{i}")
            nc.sync.dma_start(qt[:], qksw_d[P * i:P * (i + 1), :])
            qksw.append(qt)
            t = sb.tile([P, NT], BF, tag="st", bufs=CB, name=f"st{i}")
            nc.sync.dma_start(t[:], srcT_d[P * i:P * (i + 1), :])
            st.append(t)
        vsw = []
        for i in range(CB):
            vt = sb.tile([P, VW], BF, tag="vsw", bufs=CB, name=f"vsw{i}")
            nc.sync.dma_start(vt[:], vsw_d[P * i:P * (i + 1), :])
            vsw.append(vt)
        # output projection weights; W2 halves padded to K=128 with zeros.
        pjs1 = sb.tile([P, C], BF, tag="pjs1")
        nc.sync.dma_start(pjs1[:], pjsw_d[0:P, :])
        pjs2 = sb.tile([P, C], BF, tag="pjs2")
        nc.sync.dma_start(pjs2[0:DH, :], pjsw_d[P:VW, :])
        nc.gpsimd.memset(pjs2[DH:P, :], 0.0)
        pj1 = sb.tile([P, C], BF, tag="pj1")
        nc.sync.dma_start(pj1[:], pjw_d[0:P, :])
        pj2 = sb.tile([P, C], BF, tag="pj2")
        nc.sync.dma_start(pj2[0:DH, :], pjw_d[P:VW, :])
        nc.gpsimd.memset(pj2[DH:P, :], 0.0)

        # ---------------- per-head transposed QK projections (per 512 group)
        def gen_qkt_group(act_tiles, w_tiles, dst, u, q, pt):
            pst = pt(512, f"psq_u{u}_{q}")
            for cb in range(CB):
                nc.tensor.matmul(
                    pst[:],
                    lhsT=w_tiles[cb][:, D2 * u:D2 * (u + 1)],
                    rhs=act_tiles[cb][:, 512 * q:512 * (q + 1)],
                    start=(cb == 0), stop=(cb == CB - 1),
                )
            nc.vector.tensor_copy(dst[:, 512 * q:512 * (q + 1)], pst[:])

        def mk_qkt_filler(act_tiles, w_tiles, dst, u, qs_):
            def f():
                for q in qs_:
                    gen_qkt_group(act_tiles, w_tiles, dst, u, q, pt_small)
            return f

        # v in natural layout into the pre-built augmented tiles.
        def gen_vaug(act_tiles, w_tiles, vr, tag, chunks, pt=pt_small):
            for i in chunks:
                psv = pt(VW, f"psv_{tag}{i}")
                for cb in range(CB):
                    nc.tensor.matmul(
                        psv[:],
                        lhsT=act_tiles[cb][:, P * i:P * (i + 1)],
                        rhs=w_tiles[cb][:],
                        start=(cb == 0), stop=(cb == CB - 1),
                    )
                nc.vector.tensor_copy(
                    vr[:, :, i, 0:DH],
                    psv.rearrange("p (u e) -> p u e", e=DH),
                )

        # ---------------- merged attention pass (both directions, one head)
        # Software-pipelined by 2 iterations; `fillers` maps iteration index
        # (0..31, h*16+i) -> list of closures emitted at that slot.
        def head_pass(u, qx, qs, fillers, arenas_out, tail=False):
            fillers = dict(fillers or {})
            accS1 = sb.tile([65, NT], BF, tag="accS1", bufs=2,
                            name=f"accS1_{u}")
            pend = []

            def drain(n):
                while len(pend) > n:
                    pend.pop(0)()

            it = 0
            for h in range(2):
                arena_h = sb.tile([P, NAR, NT], BF, tag="arena", bufs=2,
                                  name=f"arena_{u}{h}")
                arenas_out.append(arena_h)
                acc1 = pt_a1(1024, f"acc1_{u}{h}", rows=65)
                for i in range(NCH):
                    est = sb.tile([P, 1024], BF, tag="es", bufs=6,
                                  name=f"es_{u}_{h}_{i}")
                    for q2 in range(2):
                        pst = pt_sim(512, f"ps_{u}{h}{i}{q2}")
                        lo = 1024 * h + 512 * q2
                        nc.tensor.matmul(
                            pst[:],
                            lhsT=qx[:, P * i:P * (i + 1)],
                            rhs=qs[:, lo:lo + 512],
                            start=True, stop=True,
                        )
                        nc.scalar.activation(
                            est[:, 512 * q2:512 * (q2 + 1)], pst[:],
                            mybir.ActivationFunctionType.Exp, bias=zb[:],
                        )
                    nc.sync.dma_start_transpose(
                        arena_h[:, :, P * i:P * (i + 1)], est[:])

                    def mk_acc(acc1=acc1, est=est, i=i):
                        def f():
                            vsl = vax[:, u * NCH * 65 + 65 * i:
                                      u * NCH * 65 + 65 * (i + 1)]
                            for q2 in range(2):
                                nc.tensor.matmul(
                                    acc1[:, 512 * q2:512 * (q2 + 1)],
                                    lhsT=vsl,
                                    rhs=est[:, 512 * q2:512 * (q2 + 1)],
                                    start=(i == 0), stop=(i == NCH - 1),
                                )
                        return f
                    pend.append(mk_acc())
                    drain(2)
                    for f in fillers.pop(it, []):
                        f()
                    it += 1

                def mk_close(acc1=acc1, h=h):
                    def f():
                        if tail and h == 1:
                            nc.scalar.copy(
                                accS1[:, 1024 * h:1024 * h + 512],
                                acc1[:, 0:512])
                            nc.vector.tensor_copy(
                                accS1[:, 1024 * h + 512:1024 * (h + 1)],
                                acc1[:, 512:1024])
                        else:
                            nc.vector.tensor_copy(
                                accS1[:, 1024 * h:1024 * (h + 1)], acc1[:])
                    return f
                pend.append(mk_close())
            drain(0)
            for fl in fillers.values():   # flush leftovers (shouldn't happen)
                for f in fl:
                    f()
            return accS1

        # dir-2 accumulation closures for (head u, m-half h): one closure per
        # n-half g (16 matmuls each). h==0 stashes into part2; h==1 adds.
        def mk_dir2(u, h, arena_h, part2, accS2):
            out = []
            for g in range(2):
                def f(g=g):
                    acc2 = pt_a2(1024, f"acc2_{u}{h}{g}", rows=65)
                    for j in range(NAR):
                        vsl = vas[:, u * NCH * 65 + 65 * (NAR * h + j):
                                  u * NCH * 65 + 65 * (NAR * h + j + 1)]
                        for k2 in range(2):
                            lo = 1024 * g + 512 * k2
                            nc.tensor.matmul(
                                acc2[:, 512 * k2:512 * (k2 + 1)],
                                lhsT=vsl,
                                rhs=arena_h[:, j, lo:lo + 512],
                                start=(j == 0), stop=(j == NAR - 1),
                            )
                    if h == 0:
                        nc.vector.tensor_copy(
                            part2[:, 1024 * g:1024 * (g + 1)], acc2[:])
                    else:
                        nc.vector.tensor_add(
                            accS2[:, 1024 * g:1024 * (g + 1)], acc2[:],
                            part2[:, 1024 * g:1024 * (g + 1)])
                out.append(f)
            return out

        # ---------------- normalization (psc / bb / mul), baseline machinery
        def make_norm(accS, pname, tail=False):
            state = {}

            def norm_psc():
                psc = pt_small(512, f"psc_{pname}")
                for j in range(NCH):
                    nc.tensor.matmul(
                        psc[:, j:j + 1],
                        lhsT=accS[64:65, P * j:P * (j + 1)],
                        rhs=ones64[64:65, 0:1],
                        start=True, stop=True,
                    )
                rcpF = sb.tile([P, NCH], F32, tag="rcpF", bufs=2,
                               name=f"rcpF_{pname}")
                nc.vector.reciprocal(rcpF[:], psc[:, 0:NCH])
                rcpT = sb.tile([P, NCH], BF, tag="rcpT", bufs=2,
                               name=f"rcpT_{pname}")
                nc.vector.tensor_copy(rcpT[:], rcpF[:])
                state["rcpT"] = rcpT

            def norm_bb():
                rcpT = state["rcpT"]
                bb = sb.tile([DH, NT], BF, tag="bb", bufs=2,
                             name=f"bb_{pname}")
                for k in range(N5):
                    pso = pt_small(512, f"psbb_{pname}_{k}")
                    for jj in range(4):
                        j = 4 * k + jj
                        col = rcpT[:, j:j + 1]
                        lhsT_b = bass.AP(col.tensor, col.offset,
                                         [col.ap[0], [0, DH]])
                        nc.tensor.matmul(
                            pso[0:DH, P * jj:P * (jj + 1)], lhsT=lhsT_b,
                            rhs=ident[:], start=True, stop=True,
                        )
                    if tail:  # ACT is free at the tail: split copy
                        nc.scalar.copy(bb[:, 512 * k:512 * k + 256],
                                       pso[0:DH, 0:256])
                        nc.vector.tensor_copy(
                            bb[:, 512 * k + 256:512 * (k + 1)],
                            pso[0:DH, 256:512])
                    else:
                        nc.vector.tensor_copy(bb[:, 512 * k:512 * (k + 1)],
                                              pso[0:DH, :])
                state["bb"] = bb

            def norm_mul(dst, ks=None):
                bb = state["bb"]
                if ks is None:
                    nc.vector.tensor_mul(dst, accS[0:DH, :], bb[:])
                else:
                    for k in ks:
                        sl = slice(512 * k, 512 * (k + 1))
                        nc.vector.tensor_mul(dst[:, sl], accS[0:DH, sl],
                                             bb[:, sl])
            return norm_psc, norm_bb, norm_mul

        # ---------------- partial output projection, single (cc, k) unit
        def oproj_unit(S1, S2, W1, W2, out_d, pname, cc, k, pt, split=False):
            pso = pt(512, f"pso_{pname}_{cc}_{k}")
            nc.tensor.matmul(
                pso[:], lhsT=W1[:, P * cc:P * (cc + 1)],
                rhs=S1[:, 512 * k:512 * (k + 1)],
                start=True, stop=False,
            )
            nc.tensor.matmul(
                pso[:], lhsT=W2[:, P * cc:P * (cc + 1)],
                rhs=S2[:, 512 * k:512 * (k + 1)],
                start=False, stop=True,
            )
            stg = sb.tile([P, 512], BF, tag="stg", bufs=4,
                          name=f"stg_{pname}_{cc}_{k}")
            if split:
                nc.scalar.copy(stg[:, 0:256], pso[:, 0:256])
                nc.vector.tensor_copy(stg[:, 256:512], pso[:, 256:512])
            else:
                nc.vector.tensor_copy(stg[:], pso[:])
            nc.sync.dma_start(
                out_d[P * cc:P * (cc + 1), 512 * k:512 * (k + 1)],
                stg[:],
            )

        # ---------------- lead-in: head-0 QK projections + first vax chunks
        # qkt tag rotation: qkt0 -> slot0, qkt1 -> slot1; qkt2 gets its own
        # tag (its gen is emitted while qkt0's sim reads are still pending,
        # so slot reuse would stall the gen copies).  qkst: qkst0 -> slot0,
        # qkst1 -> slot1, qkst2 -> slot0 (gen emitted in head-1, after all
        # qkst0 reads).
        qkt = [None] * NH
        qkst = [None] * NH
        qkt[0] = sb.tile([P, NT], BF, tag="qkt", bufs=2, name="qkt0")
        qkst[0] = sb.tile([P, NT], BF, tag="qkst", bufs=2, name="qkst0")

        for q in range(N5):
            gen_qkt_group(xt, qkw, qkt[0], 0, q, pt_sim)
        gen_vaug(xt, vw, vaxr, "vax", range(0, 4), pt_sim)
        for q in range(N5):
            gen_qkt_group(st, qksw, qkst[0], 0, q, pt_sim)

        # per-head dir-2 state
        part2 = [None] * NH
        accS2 = [None] * NH

        def alloc_dir2_state(u):
            part2[u] = sb.tile([65, NT], BF, tag="part2", bufs=1,
                               name=f"part2_{u}")
            accS2[u] = sb.tile([65, NT], BF, tag="accS2", bufs=1,
                               name=f"accS2_{u}")

        # dir2 closures reference arena lists that head_pass fills in as it
        # emits (arena h0 exists by the time h1's fillers run).
        def d2f(arena_list, hidx, u, g):
            def f():
                mk_dir2(u, hidx, arena_list[hidx], part2[u], accS2[u])[g]()
            return f

        # ================ head 0 ================
        alloc_dir2_state(0)
        qkt[1] = sb.tile([P, NT], BF, tag="qkt", bufs=2, name="qkt1")
        qkst[1] = sb.tile([P, NT], BF, tag="qkst", bufs=2, name="qkst1")
        qkt[2] = sb.tile([P, NT], BF, tag="qkt2x", bufs=1, name="qkt2")
        arenas0 = []
        f_vax = [
            (lambda c0: lambda: gen_vaug(xt, vw, vaxr, "vax",
                                         range(c0, c0 + 3)))(c0)
            for c0 in (4, 7, 10, 13)
        ]
        f_vas = [
            (lambda c0: lambda: gen_vaug(st, vsw, vasr, "vas",
                                         range(c0, c0 + 4)))(c0)
            for c0 in (0, 4, 8, 12)
        ]
        fills0 = {
            0: [f_vax[0]], 3: [f_vax[1]], 6: [f_vax[2]], 9: [f_vax[3]],
            11: [f_vas[0]], 14: [f_vas[1]],
            # ---- h1 (x-side gen must finish before Y stacks reuse xt slots)
            16: [f_vas[2]],
            17: [d2f(arenas0, 0, 0, 0)],
            19: [mk_qkt_filler(xt, qkw, qkt[1], 1, (0, 1))],
            21: [f_vas[3]],
            22: [d2f(arenas0, 0, 0, 1)],
            24: [mk_qkt_filler(xt, qkw, qkt[1], 1, (2, 3))],
            26: [mk_qkt_filler(st, qksw, qkst[1], 1, (0, 1))],
            28: [mk_qkt_filler(st, qksw, qkst[1], 1, (2, 3))],
            30: [mk_qkt_filler(xt, qkw, qkt[2], 2, (0, 1))],
            31: [mk_qkt_filler(xt, qkw, qkt[2], 2, (2, 3))],
        }
        accS1_0 = head_pass(0, qkt[0], qkst[0], fills0, arenas0)

        # Y / YS output stacks: reuse the xt tag slots (x-side gen done).
        Y01 = sb.tile([P, NT], BF, tag="xt", bufs=CB, name="Y01")
        Y2 = sb.tile([P, NT], BF, tag="xt", bufs=CB, name="Y2")
        YS01 = sb.tile([P, NT], BF, tag="xt", bufs=CB, name="YS01")
        YS2 = sb.tile([P, NT], BF, tag="xt", bufs=CB, name="YS2")
        nc.gpsimd.memset(Y2[DH:P, :], 0.0)
        nc.gpsimd.memset(YS2[DH:P, :], 0.0)

        def stack_slice(s01, s2, u):
            if u == 0:
                return s01[0:DH, :]
            if u == 1:
                return s01[DH:P, :]
            return s2[0:DH, :]

        # ================ head 1 ================
        alloc_dir2_state(1)
        qkst[2] = sb.tile([P, NT], BF, tag="qkst", bufs=2, name="qkst2")
        n1_0 = make_norm(accS1_0, "n1_0")
        n2_0 = make_norm(accS2[0], "n2_0")
        arenas1 = []
        fills1 = {
            0: [n1_0[0]],
            2: [d2f(arenas0, 1, 0, 0)],
            4: [n1_0[1]],
            6: [lambda: n1_0[2](stack_slice(YS01, YS2, 0))],
            8: [d2f(arenas0, 1, 0, 1)],
            10: [n2_0[0]],
            11: [mk_qkt_filler(st, qksw, qkst[2], 2, (0, 1))],
            13: [n2_0[1]],
            15: [lambda: n2_0[2](stack_slice(Y01, Y2, 0))],
            18: [d2f(arenas1, 0, 1, 0)],
            21: [mk_qkt_filler(st, qksw, qkst[2], 2, (2, 3))],
            24: [d2f(arenas1, 0, 1, 1)],
        }
        accS1_1 = head_pass(1, qkt[1], qkst[1], fills1, arenas1)

        # ================ head 2 ================
        alloc_dir2_state(2)
        n1_1 = make_norm(accS1_1, "n1_1")
        n2_1 = make_norm(accS2[1], "n2_1")
        arenas2 = []
        fills2 = {
            0: [n1_1[0]],
            2: [d2f(arenas1, 1, 1, 0)],
            4: [n1_1[1]],
            6: [lambda: n1_1[2](stack_slice(YS01, YS2, 1))],
            8: [d2f(arenas1, 1, 1, 1)],
            10: [n2_1[0]],
            13: [n2_1[1]],
            15: [lambda: n2_1[2](stack_slice(Y01, Y2, 1))],
            18: [d2f(arenas2, 0, 2, 0)],
            24: [d2f(arenas2, 0, 2, 1)],
        }
        accS1_2 = head_pass(2, qkt[2], qkst[2], fills2, arenas2,
                            tail=True)

        # ================ tail ================
        # norm1(2) -> YS complete -> YS oproj, woven with dir2(2,h1) and
        # norm2(2) -> Y complete -> Y oproj.
        n1_2 = make_norm(accS1_2, "n1_2", tail=True)
        n2_2 = make_norm(accS2[2], "n2_2", tail=True)

        n1_2[0]()
        d2f(arenas2, 1, 2, 0)()
        n1_2[1]()
        n1_2[2](stack_slice(YS01, YS2, 2), ks=range(N5))
        d2f(arenas2, 1, 2, 1)()

        units = [(cc, k) for k in range(N5) for cc in range(CB)]
        # YS oproj units woven around the norm2 chain (ACT idle -> split)
        for idx, (cc, k) in enumerate(units):
            pt = pt_a1 if idx % 2 == 0 else pt_small
            oproj_unit(YS01, YS2, pjs1, pjs2, oys_d, "osrc", cc, k, pt,
                       split=True)
            if idx == 5:
                n2_2[0]()
            elif idx == 11:
                n2_2[1]()
            elif idx == 17:
                n2_2[2](stack_slice(Y01, Y2, 2), ks=range(N5))
        for idx, (cc, k) in enumerate(units):
            pt = pt_a2 if idx % 2 == 0 else pt_a1
            oproj_unit(Y01, Y2, pj1, pj2, oy_d, "oy", cc, k, pt,
                       split=True)

    nc.compile()
    return nc


def _get_program(NT: int) -> "bacc.Bacc":
    if NT not in _PROG_CACHE:
        _PROG_CACHE[NT] = _build_program(NT)
    return _PROG_CACHE[NT]


def make_in_maps(x, src, qk_w, qk_src_w, v_w, v_src_w, proj_w, proj_src_w):
    """Host-side sharding: per-core input dicts (pure data marshalling)."""
    bf = ml_dtypes.bfloat16

    def prep(a):
        return np.ascontiguousarray(a).astype(bf)

    in_maps = []
    for c in range(N_CORES):
        b = c // 4
        heads = [3 * (c % 4) + j for j in range(NH)]
        qk_rows = np.concatenate([qk_w[D2 * h:D2 * (h + 1), :] for h in heads])
        qks_rows = np.concatenate(
            [qk_src_w[D2 * h:D2 * (h + 1), :] for h in heads])
        v_rows = np.concatenate([v_w[DH * h:DH * (h + 1), :] for h in heads])
        vs_rows = np.concatenate(
            [v_src_w[DH * h:DH * (h + 1), :] for h in heads])
        pj_cols = np.concatenate(
            [proj_w[:, DH * h:DH * (h + 1)] for h in heads], axis=1)
        pjs_cols = np.concatenate(
            [proj_src_w[:, DH * h:DH * (h + 1)] for h in heads], axis=1)
        in_maps.append({
            "ident": np.eye(P).astype(ml_dtypes.bfloat16),
            "xT": prep(x[b].T),
            "srcT": prep(src[b].T),
            "qk_wT": prep(qk_rows.T * SCALE),
            "qks_wT": prep(qks_rows.T),
            "v_wT": prep(v_rows.T),
            "vs_wT": prep(vs_rows.T),
            "projT": prep(pj_cols.T),
            "projsT": prep(pjs_cols.T),
        })
    return in_maps


LAST_RESULTS = None  # BassKernelResults of the most recent kernel() call
_HOOK_DONE = False


def _install_ntff_hook():
    """The agent image's antenv lacks axon_hooks; inject a stub module and
    register the ctypes NTFF profile hook so trace=True yields exec times."""
    global _HOOK_DONE
    if _HOOK_DONE:
        return
    try:
        import types
        import antenv  # noqa: F401
        if "antenv.axon_hooks" not in sys.modules:
            mod = types.ModuleType("antenv.axon_hooks")
            _hook = [None]
            mod.set_axon_ntff_profile_hook = lambda h: _hook.__setitem__(0, h)
            mod.get_axon_ntff_profile_hook = lambda: _hook[0]
            sys.modules["antenv.axon_hooks"] = mod
        import trn_agent_boot.trn_boot as _tb
        from antenv.axon_hooks import set_axon_ntff_profile_hook
        set_axon_ntff_profile_hook(
            _tb._ntff_profile_via_ctypes("/opt/axon/libaxon_pjrt.so"))
        _HOOK_DONE = True
    except Exception as e:  # profiling is best-effort
        print(f"ntff hook install failed: {e}", file=sys.stderr)


def kernel(x, src, qk_w, qk_src_w, v_w, v_src_w, proj_w, proj_b,
           proj_src_w, proj_src_b):
    global LAST_RESULTS
    x = np.asarray(x, np.float32)
    src = np.asarray(src, np.float32)
    NT = x.shape[1]

    in_maps = make_in_maps(
        x, src,
        np.asarray(qk_w, np.float32), np.asarray(qk_src_w, np.float32),
        np.asarray(v_w, np.float32), np.asarray(v_src_w, np.float32),
        np.asarray(proj_w, np.float32), np.asarray(proj_src_w, np.float32),
    )

    nc = _get_program(NT)
    trace = bool(int(os.environ.get("BCA_TRACE", "0")))
    if trace:
        _install_ntff_hook()
    res = bass_utils.run_bass_kernel_spmd(
        nc, in_maps, core_ids=list(range(N_CORES)), trace=trace,
    )
    LAST_RESULTS = res

    # host gather: sum partial projections over the 4 cores of each batch,
    # transpose back, add biases, concat branches.
    oy = np.zeros((B, NT, C), np.float32)
    oys = np.zeros((B, NT, C), np.float32)
    for c in range(N_CORES):
        b = c // 4
        oy[b] += np.asarray(res.results[c]["out_y"], np.float32).T
        oys[b] += np.asarray(res.results[c]["out_ys"], np.float32).T
    oy += np.asarray(proj_b, np.float32)
    oys += np.asarray(proj_src_b, np.float32)
    return np.concatenate([oy, oys], axis=-1).astype(np.float32)


# revision 15
# speedup vs baseline: 1.0988x; 1.0235x over previous
"""Bidirectional cross-attention Trainium2 kernel (v3).

Sharding: (batch, head) units. B=2, H=12 -> 24 units over 8 cores:
core c handles batch b = c // 4 and heads 3*(c%4) .. 3*(c%4)+2.
Each core computes the full attention for its 3 heads plus the partial
output projections; the host sums the per-core partial projections,
transposes back, adds biases and concatenates the two branches.

v3 restructure vs v2: each head's two softmax directions share ONE sim
computation. exp(sim) tiles are transposed SBUF->SBUF by the DMA XBAR
(16x128 tiles; runs on otherwise-idle DMA engines) into a per-half
arena laid out [m-chunk(8), m(128), n(2048)]; the second direction's
accumulation consumes the arena directly. This removes the per-head
sim recompute (~33k PE cycles/head) and halves ACT exp work.

Schedule:
- One merged pass per head, two m-halves of 1024 cols each. Per
  iteration (n-chunk i): 2x sim matmul [128,512] -> exp -> est tile;
  one XBAR-transpose DMA per est tile into the arena; dir-1 acc
  matmuls pipelined 2 iterations behind (pend/drain).
- dir-2 acc (16 matmuls per (half, n-half)) is woven as fillers into
  the NEXT phase's iteration stream; halves are combined via a bf16
  SBUF partial (part2) + DVE add.
- PSUM: ps_sim 3x[128,512] + ps_acc1 [128,1024] + ps_acc2 [128,1024]
  + ps_small [128,512] = 8 banks.
- Y01/Y2/YS01/YS2 output stacks reuse the xt tag's SBUF slots (all
  x-side gen is emitted before the stack allocs), freeing room for
  the 2x4MB arena.
"""

import os
import sys
from contextlib import ExitStack

import numpy as np

sys.path.insert(0, "/opt/trn_rl_repo")

import ml_dtypes  # noqa: E402

import concourse.bass as bass  # noqa: E402
import concourse.tile as tile  # noqa: E402
from concourse import bacc, mybir  # noqa: E402
from concourse import bass_utils  # noqa: E402

# ---------------------------------------------------------------- constants
P = 128          # partitions
C = 768          # channels
CB = C // P      # 6 channel blocks
NH = 3           # heads per core
D2 = 128         # qk dims per head (2*HEAD_DIM)
DH = 64          # v dims per head
QW = NH * D2     # 384
VW = NH * DH     # 192
H = 12
B = 2
N_CORES = 8
SCALE = DH ** -0.5

BF = mybir.dt.bfloat16
F32 = mybir.dt.float32

_PROG_CACHE: dict[int, "bacc.Bacc"] = {}


def _build_program(NT: int) -> "bacc.Bacc":
    """Build+schedule+compile the per-core Bass program (SPMD: same program
    on all 8 cores, per-core data differs)."""
    NCH = NT // P      # 128-row chunks (16)
    N5 = NT // 512     # 512-col groups (4)
    NAR = NT // 256    # arena m-blocks per half (8)

    nc = bacc.Bacc(
        "TRN2",
        target_bir_lowering=False,
        debug=False,
        num_devices=N_CORES,
    )

    xT_d = nc.dram_tensor("xT", [C, NT], BF, kind="ExternalInput").ap()
    srcT_d = nc.dram_tensor("srcT", [C, NT], BF, kind="ExternalInput").ap()
    qkw_d = nc.dram_tensor("qk_wT", [C, QW], BF, kind="ExternalInput").ap()
    qksw_d = nc.dram_tensor("qks_wT", [C, QW], BF, kind="ExternalInput").ap()
    vw_d = nc.dram_tensor("v_wT", [C, VW], BF, kind="ExternalInput").ap()
    vsw_d = nc.dram_tensor("vs_wT", [C, VW], BF, kind="ExternalInput").ap()
    pjw_d = nc.dram_tensor("projT", [VW, C], BF, kind="ExternalInput").ap()
    pjsw_d = nc.dram_tensor("projsT", [VW, C], BF, kind="ExternalInput").ap()
    ident_d = nc.dram_tensor("ident", [P, P], BF, kind="ExternalInput").ap()
    oy_d = nc.dram_tensor("out_y", [C, NT], BF, kind="ExternalOutput").ap()
    oys_d = nc.dram_tensor("out_ys", [C, NT], BF, kind="ExternalOutput").ap()

    with tile.TileContext(nc) as tc, ExitStack() as ctx:
        sb = ctx.enter_context(tc.tile_pool(name="sb", bufs=1, space="SBUF"))
        ps = ctx.enter_context(tc.tile_pool(name="ps", bufs=1, space="PSUM"))

        def pt_sim(cols, name):
            return ps.tile([P, cols], F32, tag="ps_sim", bufs=3, name=name,
                           padded_shape=[P, 512])

        def pt_a1(cols, name, rows=P):
            return ps.tile([rows, cols], F32, tag="ps_acc1", bufs=1,
                           name=name, padded_shape=[P, 1024])

        def pt_a2(cols, name, rows=P):
            return ps.tile([rows, cols], F32, tag="ps_acc2", bufs=1,
                           name=name, padded_shape=[P, 1024])

        def pt_small(cols, name):
            return ps.tile([P, cols], F32, tag="ps_small", bufs=1, name=name,
                           padded_shape=[P, 512])

        # ---------------- constants / zero-fills (no DRAM deps)
        ones64 = sb.tile([P, DH], BF, tag="ones64")
        nc.gpsimd.memset(ones64[:], 1.0)
        zb = sb.tile([P, 1], F32, tag="zb")
        nc.gpsimd.memset(zb[:], 0.0)
        ident = sb.tile([P, P], BF, tag="ident")
        nc.sync.dma_start(ident[:], ident_d[:])

        # v tensors, ones-augmented: va[:, u*NCH*65 + 65*i + e] with
        # e in [0,64) = v values, e == 64 = 1.0 (softmax denominator row).
        vax = sb.tile([P, NH * NCH * 65], BF, tag="vax", name="vax")
        vas = sb.tile([P, NH * NCH * 65], BF, tag="vas", name="vas")
        vaxr = vax.rearrange("p (u i e) -> p u i e", u=NH, e=65)
        vasr = vas.rearrange("p (u i e) -> p u i e", u=NH, e=65)
        nc.gpsimd.memset(vaxr[:, :, :, DH:65], 1.0)
        nc.gpsimd.memset(vasr[:, :, :, DH:65], 1.0)

        # ---------------- input loads (emission order == DMA priority)
        xt, qkw, vw = [], [], []
        for i in range(CB):
            qt = sb.tile([P, QW], BF, tag="qkw", bufs=CB, name=f"qkw{i}")
            nc.sync.dma_start(qt[:], qkw_d[P * i:P * (i + 1), :])
            qkw.append(qt)
            vt = sb.tile([P, VW], BF, tag="vw", bufs=CB, name=f"vw{i}")
            nc.sync.dma_start(vt[:], vw_d[P * i:P * (i + 1), :])
            vw.append(vt)
            t = sb.tile([P, NT], BF, tag="xt", bufs=CB, name=f"xt{i}")
            nc.sync.dma_start(t[:], xT_d[P * i:P * (i + 1), :])
            xt.append(t)
        st, qksw = [], []
        for i in range(CB):
            qt = sb.tile([P, QW], BF, tag="qksw", bufs=CB, name=f"qksw{i}")
            nc.sync.dma_start(qt[:], qksw_d[P * i:P * (i + 1), :])
            qksw.append(qt)
            t = sb.tile([P, NT], BF, tag="st", bufs=CB, name=f"st{i}")
            nc.sync.dma_start(t[:], srcT_d[P * i:P * (i + 1), :])
            st.append(t)
        vsw = []
        for i in range(CB):
            vt = sb.tile([P, VW], BF, tag="vsw", bufs=CB, name=f"vsw{i}")
            nc.sync.dma_start(vt[:], vsw_d[P * i:P * (i + 1), :])
            vsw.append(vt)
        # output projection weights; W2 halves padded to K=128 with zeros.
        pjs1 = sb.tile([P, C], BF, tag="pjs1")
        nc.sync.dma_start(pjs1[:], pjsw_d[0:P, :])
        pjs2 = sb.tile([P, C], BF, tag="pjs2")
        nc.sync.dma_start(pjs2[0:DH, :], pjsw_d[P:VW, :])
        nc.gpsimd.memset(pjs2[DH:P, :], 0.0)
        pj1 = sb.tile([P, C], BF, tag="pj1")
        nc.sync.dma_start(pj1[:], pjw_d[0:P, :])
        pj2 = sb.tile([P, C], BF, tag="pj2")
        nc.sync.dma_start(pj2[0:DH, :], pjw_d[P:VW, :])
        nc.gpsimd.memset(pj2[DH:P, :], 0.0)

        # ---------------- per-head transposed QK projections (per 512 group)
        def gen_qkt_group(act_tiles, w_tiles, dst, u, q, pt):
            pst = pt(512, f"psq_u{u}_{q}")
            for cb in range(CB):
                nc.tensor.matmul(
                    pst[:],
                    lhsT=w_tiles[cb][:, D2 * u:D2 * (u + 1)],
                    rhs=act_tiles[cb][:, 512 * q:512 * (q + 1)],
                    start=(cb == 0), stop=(cb == CB - 1),
                )
            nc.vector.tensor_copy(dst[:, 512 * q:512 * (q + 1)], pst[:])

        def mk_qkt_filler(act_tiles, w_tiles, dst, u, qs_):
            def f():
                for q in qs_:
                    gen_qkt_group(act_tiles, w_tiles, dst, u, q, pt_small)
            return f

        # v in natural layout into the pre-built augmented tiles.
        def gen_vaug(act_tiles, w_tiles, vr, tag, chunks, pt=pt_small):
            for i in chunks:
                psv = pt(VW, f"psv_{tag}{i}")
                for cb in range(CB):
                    nc.tensor.matmul(
                        psv[:],
                        lhsT=act_tiles[cb][:, P * i:P * (i + 1)],
                        rhs=w_tiles[cb][:],
                        start=(cb == 0), stop=(cb == CB - 1),
                    )
                nc.vector.tensor_copy(
                    vr[:, :, i, 0:DH],
                    psv.rearrange("p (u e) -> p u e", e=DH),
                )

        # ---------------- merged attention pass (both directions, one head)
        # Software-pipelined by 2 iterations; `fillers` maps iteration index
        # (0..31, h*16+i) -> list of closures emitted at that slot.
        def head_pass(u, qx, qs, fillers, arenas_out, tail=False):
            fillers = dict(fillers or {})
            accS1 = sb.tile([65, NT], BF, tag="accS1", bufs=2,
                            name=f"accS1_{u}")
            pend = []

            def drain(n):
                while len(pend) > n:
                    pend.pop(0)()

            it = 0
            for h in range(2):
                arena_h = sb.tile([P, NAR, NT], BF, tag="arena", bufs=2,
                                  name=f"arena_{u}{h}")
                arenas_out.append(arena_h)
                acc1 = pt_a1(1024, f"acc1_{u}{h}", rows=65)
                for i in range(NCH):
                    est = sb.tile([P, 1024], BF, tag="es", bufs=6,
                                  name=f"es_{u}_{h}_{i}")
                    for q2 in range(2):
                        pst = pt_sim(512, f"ps_{u}{h}{i}{q2}")
                        lo = 1024 * h + 512 * q2
                        nc.tensor.matmul(
                            pst[:],
                            lhsT=qx[:, P * i:P * (i + 1)],
                            rhs=qs[:, lo:lo + 512],
                            start=True, stop=True,
                        )
                        nc.scalar.activation(
                            est[:, 512 * q2:512 * (q2 + 1)], pst[:],
                            mybir.ActivationFunctionType.Exp, bias=zb[:],
                        )
                    # alternate DGE rings (SP / Activation) so neither
                    # descriptor ring saturates on the XBAR packet flood
                    dge = nc.sync if i % 2 == 0 else nc.scalar
                    dge.dma_start_transpose(
                        arena_h[:, :, P * i:P * (i + 1)], est[:])

                    def mk_acc(acc1=acc1, est=est, i=i):
                        def f():
                            vsl = vax[:, u * NCH * 65 + 65 * i:
                                      u * NCH * 65 + 65 * (i + 1)]
                            for q2 in range(2):
                                nc.tensor.matmul(
                                    acc1[:, 512 * q2:512 * (q2 + 1)],
                                    lhsT=vsl,
                                    rhs=est[:, 512 * q2:512 * (q2 + 1)],
                                    start=(i == 0), stop=(i == NCH - 1),
                                )
                        return f
                    pend.append(mk_acc())
                    drain(2)
                    for f in fillers.pop(it, []):
                        f()
                    it += 1

                def mk_close(acc1=acc1, h=h):
                    def f():
                        if tail and h == 1:
                            nc.scalar.copy(
                                accS1[:, 1024 * h:1024 * h + 512],
                                acc1[:, 0:512])
                            nc.vector.tensor_copy(
                                accS1[:, 1024 * h + 512:1024 * (h + 1)],
                                acc1[:, 512:1024])
                        else:
                            nc.vector.tensor_copy(
                                accS1[:, 1024 * h:1024 * (h + 1)], acc1[:])
                    return f
                pend.append(mk_close())
            drain(0)
            for fl in fillers.values():   # flush leftovers (shouldn't happen)
                for f in fl:
                    f()
            return accS1

        # dir-2 accumulation closures for (head u, m-half h): one closure per
        # n-half g (16 matmuls each). h==0 stashes into part2; h==1 adds.
        def mk_dir2(u, h, arena_h, part2, accS2):
            out = []
            for g in range(2):
                def f(g=g):
                    acc2 = pt_a2(1024, f"acc2_{u}{h}{g}", rows=65)
                    for j in range(NAR):
                        vsl = vas[:, u * NCH * 65 + 65 * (NAR * h + j):
                                  u * NCH * 65 + 65 * (NAR * h + j + 1)]
                        for k2 in range(2):
                            lo = 1024 * g + 512 * k2
                            nc.tensor.matmul(
                                acc2[:, 512 * k2:512 * (k2 + 1)],
                                lhsT=vsl,
                                rhs=arena_h[:, j, lo:lo + 512],
                                start=(j == 0), stop=(j == NAR - 1),
                            )
                    if h == 0:
                        nc.vector.tensor_copy(
                            part2[:, 1024 * g:1024 * (g + 1)], acc2[:])
                    else:
                        nc.vector.tensor_add(
                            accS2[:, 1024 * g:1024 * (g + 1)], acc2[:],
                            part2[:, 1024 * g:1024 * (g + 1)])
                out.append(f)
            return out

        # ---------------- normalization (psc / bb / mul), baseline machinery
        def make_norm(accS, pname, tail=False):
            state = {}

            def norm_psc():
                psc = pt_small(512, f"psc_{pname}")
                for j in range(NCH):
                    nc.tensor.matmul(
                        psc[:, j:j + 1],
                        lhsT=accS[64:65, P * j:P * (j + 1)],
                        rhs=ones64[64:65, 0:1],
                        start=True, stop=True,
                    )
                rcpF = sb.tile([P, NCH], F32, tag="rcpF", bufs=2,
                               name=f"rcpF_{pname}")
                nc.vector.reciprocal(rcpF[:], psc[:, 0:NCH])
                rcpT = sb.tile([P, NCH], BF, tag="rcpT", bufs=2,
                               name=f"rcpT_{pname}")
                nc.vector.tensor_copy(rcpT[:], rcpF[:])
                state["rcpT"] = rcpT

            def norm_bb():
                rcpT = state["rcpT"]
                bb = sb.tile([DH, NT], BF, tag="bb", bufs=2,
                             name=f"bb_{pname}")
                for k in range(N5):
                    pso = pt_small(512, f"psbb_{pname}_{k}")
                    for jj in range(4):
                        j = 4 * k + jj
                        col = rcpT[:, j:j + 1]
                        lhsT_b = bass.AP(col.tensor, col.offset,
                                         [col.ap[0], [0, DH]])
                        nc.tensor.matmul(
                            pso[0:DH, P * jj:P * (jj + 1)], lhsT=lhsT_b,
                            rhs=ident[:], start=True, stop=True,
                        )
                    if tail:  # ACT is free at the tail: split copy
                        nc.scalar.copy(bb[:, 512 * k:512 * k + 256],
                                       pso[0:DH, 0:256])
                        nc.vector.tensor_copy(
                            bb[:, 512 * k + 256:512 * (k + 1)],
                            pso[0:DH, 256:512])
                    else:
                        nc.vector.tensor_copy(bb[:, 512 * k:512 * (k + 1)],
                                              pso[0:DH, :])
                state["bb"] = bb

            def norm_mul(dst, ks=None):
                bb = state["bb"]
                if ks is None:
                    nc.vector.tensor_mul(dst, accS[0:DH, :], bb[:])
                else:
                    for k in ks:
                        sl = slice(512 * k, 512 * (k + 1))
                        nc.vector.tensor_mul(dst[:, sl], accS[0:DH, sl],
                                             bb[:, sl])
            return norm_psc, norm_bb, norm_mul

        # ---------------- partial output projection, single (cc, k) unit
        def oproj_unit(S1, S2, W1, W2, out_d, pname, cc, k, pt, split=False,
                       dge=None):
            pso = pt(512, f"pso_{pname}_{cc}_{k}")
            nc.tensor.matmul(
                pso[:], lhsT=W1[:, P * cc:P * (cc + 1)],
                rhs=S1[:, 512 * k:512 * (k + 1)],
                start=True, stop=False,
            )
            nc.tensor.matmul(
                pso[:], lhsT=W2[:, P * cc:P * (cc + 1)],
                rhs=S2[:, 512 * k:512 * (k + 1)],
                start=False, stop=True,
            )
            stg = sb.tile([P, 512], BF, tag="stg", bufs=6,
                          name=f"stg_{pname}_{cc}_{k}")
            if split:
                nc.scalar.copy(stg[:, 0:256], pso[:, 0:256])
                nc.vector.tensor_copy(stg[:, 256:512], pso[:, 256:512])
            else:
                nc.vector.tensor_copy(stg[:], pso[:])
            (dge or nc.sync).dma_start(
                out_d[P * cc:P * (cc + 1), 512 * k:512 * (k + 1)],
                stg[:],
            )

        # ---------------- lead-in: head-0 QK projections + first vax chunks
        # qkt tag rotation: qkt0 -> slot0, qkt1 -> slot1; qkt2 gets its own
        # tag (its gen is emitted while qkt0's sim reads are still pending,
        # so slot reuse would stall the gen copies).  qkst: qkst0 -> slot0,
        # qkst1 -> slot1, qkst2 -> slot0 (gen emitted in head-1, after all
        # qkst0 reads).
        qkt = [None] * NH
        qkst = [None] * NH
        qkt[0] = sb.tile([P, NT], BF, tag="qkt", bufs=2, name="qkt0")
        qkst[0] = sb.tile([P, NT], BF, tag="qkst", bufs=2, name="qkst0")

        for q in range(N5):
            gen_qkt_group(xt, qkw, qkt[0], 0, q, pt_sim)
        gen_vaug(xt, vw, vaxr, "vax", range(0, 4), pt_sim)
        for q in range(N5):
            gen_qkt_group(st, qksw, qkst[0], 0, q, pt_sim)

        # per-head dir-2 state
        part2 = [None] * NH
        accS2 = [None] * NH

        def alloc_dir2_state(u):
            part2[u] = sb.tile([65, NT], BF, tag="part2", bufs=1,
                               name=f"part2_{u}")
            accS2[u] = sb.tile([65, NT], BF, tag="accS2", bufs=1,
                               name=f"accS2_{u}")

        # dir2 closures reference arena lists that head_pass fills in as it
        # emits (arena h0 exists by the time h1's fillers run).
        def d2f(arena_list, hidx, u, g):
            def f():
                mk_dir2(u, hidx, arena_list[hidx], part2[u], accS2[u])[g]()
            return f

        # ================ head 0 ================
        alloc_dir2_state(0)
        qkt[1] = sb.tile([P, NT], BF, tag="qkt", bufs=2, name="qkt1")
        qkst[1] = sb.tile([P, NT], BF, tag="qkst", bufs=2, name="qkst1")
        qkt[2] = sb.tile([P, NT], BF, tag="qkt2x", bufs=1, name="qkt2")
        arenas0 = []
        f_vax = [
            (lambda c0: lambda: gen_vaug(xt, vw, vaxr, "vax",
                                         range(c0, c0 + 3)))(c0)
            for c0 in (4, 7, 10, 13)
        ]
        f_vas = [
            (lambda c0: lambda: gen_vaug(st, vsw, vasr, "vas",
                                         range(c0, c0 + 4)))(c0)
            for c0 in (0, 4, 8, 12)
        ]
        fills0 = {
            0: [f_vax[0]], 3: [f_vax[1]], 6: [f_vax[2]], 9: [f_vax[3]],
            11: [f_vas[0]], 14: [f_vas[1]],
            # ---- h1 (x-side gen must finish before Y stacks reuse xt slots)
            16: [f_vas[2]],
            17: [d2f(arenas0, 0, 0, 0)],
            19: [mk_qkt_filler(xt, qkw, qkt[1], 1, (0, 1))],
            21: [f_vas[3]],
            22: [d2f(arenas0, 0, 0, 1)],
            24: [mk_qkt_filler(xt, qkw, qkt[1], 1, (2, 3))],
            26: [mk_qkt_filler(st, qksw, qkst[1], 1, (0, 1))],
            28: [mk_qkt_filler(st, qksw, qkst[1], 1, (2, 3))],
            30: [mk_qkt_filler(xt, qkw, qkt[2], 2, (0, 1))],
            31: [mk_qkt_filler(xt, qkw, qkt[2], 2, (2, 3))],
        }
        accS1_0 = head_pass(0, qkt[0], qkst[0], fills0, arenas0)

        # Y / YS output stacks: reuse the xt tag slots (x-side gen done).
        Y01 = sb.tile([P, NT], BF, tag="xt", bufs=CB, name="Y01")
        Y2 = sb.tile([P, NT], BF, tag="xt", bufs=CB, name="Y2")
        YS01 = sb.tile([P, NT], BF, tag="xt", bufs=CB, name="YS01")
        YS2 = sb.tile([P, NT], BF, tag="xt", bufs=CB, name="YS2")
        nc.gpsimd.memset(Y2[DH:P, :], 0.0)
        nc.gpsimd.memset(YS2[DH:P, :], 0.0)

        def stack_slice(s01, s2, u):
            if u == 0:
                return s01[0:DH, :]
            if u == 1:
                return s01[DH:P, :]
            return s2[0:DH, :]

        # ================ head 1 ================
        alloc_dir2_state(1)
        qkst[2] = sb.tile([P, NT], BF, tag="qkst", bufs=2, name="qkst2")
        n1_0 = make_norm(accS1_0, "n1_0")
        n2_0 = make_norm(accS2[0], "n2_0")
        arenas1 = []
        fills1 = {
            0: [n1_0[0]],
            2: [d2f(arenas0, 1, 0, 0)],
            4: [n1_0[1]],
            6: [lambda: n1_0[2](stack_slice(YS01, YS2, 0))],
            8: [d2f(arenas0, 1, 0, 1)],
            10: [n2_0[0]],
            11: [mk_qkt_filler(st, qksw, qkst[2], 2, (0, 1))],
            13: [n2_0[1]],
            15: [lambda: n2_0[2](stack_slice(Y01, Y2, 0))],
            18: [d2f(arenas1, 0, 1, 0)],
            21: [mk_qkt_filler(st, qksw, qkst[2], 2, (2, 3))],
            24: [d2f(arenas1, 0, 1, 1)],
        }
        accS1_1 = head_pass(1, qkt[1], qkst[1], fills1, arenas1)

        # ================ head 2 ================
        alloc_dir2_state(2)
        n1_1 = make_norm(accS1_1, "n1_1")
        n2_1 = make_norm(accS2[1], "n2_1")
        arenas2 = []
        fills2 = {
            0: [n1_1[0]],
            2: [d2f(arenas1, 1, 1, 0)],
            4: [n1_1[1]],
            6: [lambda: n1_1[2](stack_slice(YS01, YS2, 1))],
            8: [d2f(arenas1, 1, 1, 1)],
            10: [n2_1[0]],
            13: [n2_1[1]],
            15: [lambda: n2_1[2](stack_slice(Y01, Y2, 1))],
            18: [d2f(arenas2, 0, 2, 0)],
            24: [d2f(arenas2, 0, 2, 1)],
        }
        accS1_2 = head_pass(2, qkt[2], qkst[2], fills2, arenas2,
                            tail=True)

        # ================ tail ================
        # norm1(2) -> YS complete -> YS oproj, woven with dir2(2,h1) and
        # norm2(2) -> Y complete -> Y oproj.
        n1_2 = make_norm(accS1_2, "n1_2", tail=True)
        n2_2 = make_norm(accS2[2], "n2_2", tail=True)

        n1_2[0]()
        d2f(arenas2, 1, 2, 0)()
        n1_2[1]()
        n1_2[2](stack_slice(YS01, YS2, 2), ks=range(N5))
        d2f(arenas2, 1, 2, 1)()

        units = [(cc, k) for k in range(N5) for cc in range(CB)]
        # YS oproj units woven around the norm2 chain; rotate 3 psum slots
        # deep (ps_sim slots are idle after the last sim matmul) and
        # alternate store-DMA rings so neither dispatcher backs up.
        ys_rot = [pt_a1, pt_sim, pt_small]
        for idx, (cc, k) in enumerate(units):
            oproj_unit(YS01, YS2, pjs1, pjs2, oys_d, "osrc", cc, k,
                       ys_rot[idx % 3], split=True,
                       dge=nc.sync if idx % 2 == 0 else nc.scalar)
            if idx == 5:
                n2_2[0]()
            elif idx == 11:
                n2_2[1]()
            elif idx == 17:
                n2_2[2](stack_slice(Y01, Y2, 2), ks=range(N5))
        y_rot = [pt_a2, pt_sim, pt_a1, pt_small]
        for idx, (cc, k) in enumerate(units):
            oproj_unit(Y01, Y2, pj1, pj2, oy_d, "oy", cc, k,
                       y_rot[idx % 4], split=True,
                       dge=nc.sync if idx % 2 == 0 else nc.scalar)

    nc.compile()
    return nc


def _get_program(NT: int) -> "bacc.Bacc":
    if NT not in _PROG_CACHE:
        _PROG_CACHE[NT] = _build_program(NT)
    return _PROG_CACHE[NT]


def make_in_maps(x, src, qk_w, qk_src_w, v_w, v_src_w, proj_w, proj_src_w):
    """Host-side sharding: per-core input dicts (pure data marshalling)."""
    bf = ml_dtypes.bfloat16

    def prep(a):
        return np.ascontiguousarray(a).astype(bf)

    in_maps = []
    for c in range(N_CORES):
        b = c // 4
        heads = [3 * (c % 4) + j for j in range(NH)]
        qk_rows = np.concatenate([qk_w[D2 * h:D2 * (h + 1), :] for h in heads])
        qks_rows = np.concatenate(
            [qk_src_w[D2 * h:D2 * (h + 1), :] for h in heads])
        v_rows = np.concatenate([v_w[DH * h:DH * (h + 1), :] for h in heads])
        vs_rows = np.concatenate(
            [v_src_w[DH * h:DH * (h + 1), :] for h in heads])
        pj_cols = np.concatenate(
            [proj_w[:, DH * h:DH * (h + 1)] for h in heads], axis=1)
        pjs_cols = np.concatenate(
            [proj_src_w[:, DH * h:DH * (h + 1)] for h in heads], axis=1)
        in_maps.append({
            "ident": np.eye(P).astype(ml_dtypes.bfloat16),
            "xT": prep(x[b].T),
            "srcT": prep(src[b].T),
            "qk_wT": prep(qk_rows.T * SCALE),
            "qks_wT": prep(qks_rows.T),
            "v_wT": prep(v_rows.T),
            "vs_wT": prep(vs_rows.T),
            "projT": prep(pj_cols.T),
            "projsT": prep(pjs_cols.T),
        })
    return in_maps


LAST_RESULTS = None  # BassKernelResults of the most recent kernel() call
_HOOK_DONE = False


def _install_ntff_hook():
    """The agent image's antenv lacks axon_hooks; inject a stub module and
    register the ctypes NTFF profile hook so trace=True yields exec times."""
    global _HOOK_DONE
    if _HOOK_DONE:
        return
    try:
        import types
        import antenv  # noqa: F401
        if "antenv.axon_hooks" not in sys.modules:
            mod = types.ModuleType("antenv.axon_hooks")
            _hook = [None]
            mod.set_axon_ntff_profile_hook = lambda h: _hook.__setitem__(0, h)
            mod.get_axon_ntff_profile_hook = lambda: _hook[0]
            sys.modules["antenv.axon_hooks"] = mod
        import trn_agent_boot.trn_boot as _tb
        from antenv.axon_hooks import set_axon_ntff_profile_hook
        set_axon_ntff_profile_hook(
            _tb._ntff_profile_via_ctypes("/opt/axon/libaxon_pjrt.so"))
        _HOOK_DONE = True
    except Exception as e:  # profiling is best-effort
        print(f"ntff hook install failed: {e}", file=sys.stderr)


def kernel(x, src, qk_w, qk_src_w, v_w, v_src_w, proj_w, proj_b,
           proj_src_w, proj_src_b):
    global LAST_RESULTS
    x = np.asarray(x, np.float32)
    src = np.asarray(src, np.float32)
    NT = x.shape[1]

    in_maps = make_in_maps(
        x, src,
        np.asarray(qk_w, np.float32), np.asarray(qk_src_w, np.float32),
        np.asarray(v_w, np.float32), np.asarray(v_src_w, np.float32),
        np.asarray(proj_w, np.float32), np.asarray(proj_src_w, np.float32),
    )

    nc = _get_program(NT)
    trace = bool(int(os.environ.get("BCA_TRACE", "0")))
    if trace:
        _install_ntff_hook()
    res = bass_utils.run_bass_kernel_spmd(
        nc, in_maps, core_ids=list(range(N_CORES)), trace=trace,
    )
    LAST_RESULTS = res

    # host gather: sum partial projections over the 4 cores of each batch,
    # transpose back, add biases, concat branches.
    oy = np.zeros((B, NT, C), np.float32)
    oys = np.zeros((B, NT, C), np.float32)
    for c in range(N_CORES):
        b = c // 4
        oy[b] += np.asarray(res.results[c]["out_y"], np.float32).T
        oys[b] += np.asarray(res.results[c]["out_ys"], np.float32).T
    oy += np.asarray(proj_b, np.float32)
    oys += np.asarray(proj_src_b, np.float32)
    return np.concatenate([oy, oys], axis=-1).astype(np.float32)


# revision 17
# speedup vs baseline: 1.1533x; 1.0496x over previous
"""Bidirectional cross-attention Trainium2 kernel (v3).

Sharding: (batch, head) units. B=2, H=12 -> 24 units over 8 cores:
core c handles batch b = c // 4 and heads 3*(c%4) .. 3*(c%4)+2.
Each core computes the full attention for its 3 heads plus the partial
output projections; the host sums the per-core partial projections,
transposes back, adds biases and concatenates the two branches.

v3 restructure vs v2: each head's two softmax directions share ONE sim
computation. exp(sim) tiles are transposed SBUF->SBUF by the DMA XBAR
(16x128 tiles; runs on otherwise-idle DMA engines) into a per-half
arena laid out [m-chunk(8), m(128), n(2048)]; the second direction's
accumulation consumes the arena directly. This removes the per-head
sim recompute (~33k PE cycles/head) and halves ACT exp work.

Schedule:
- One merged pass per head, two m-halves of 1024 cols each. Per
  iteration (n-chunk i): 2x sim matmul [128,512] -> exp -> est tile;
  one XBAR-transpose DMA per est tile into the arena; dir-1 acc
  matmuls pipelined 2 iterations behind (pend/drain).
- dir-2 acc (16 matmuls per (half, n-half)) is woven as fillers into
  the NEXT phase's iteration stream; halves are combined via a bf16
  SBUF partial (part2) + DVE add.
- PSUM: ps_sim 3x[128,512] + ps_acc1 [128,1024] + ps_acc2 [128,1024]
  + ps_small [128,512] = 8 banks.
- Y01/Y2/YS01/YS2 output stacks reuse the xt tag's SBUF slots (all
  x-side gen is emitted before the stack allocs), freeing room for
  the 2x4MB arena.
"""

import os
import sys
from contextlib import ExitStack

import numpy as np

sys.path.insert(0, "/opt/trn_rl_repo")

import ml_dtypes  # noqa: E402

import concourse.bass as bass  # noqa: E402
import concourse.tile as tile  # noqa: E402
from concourse import bacc, mybir  # noqa: E402
from concourse import bass_utils  # noqa: E402

# ---------------------------------------------------------------- constants
P = 128          # partitions
C = 768          # channels
CB = C // P      # 6 channel blocks
NH = 3           # heads per core
D2 = 128         # qk dims per head (2*HEAD_DIM)
DH = 64          # v dims per head
QW = NH * D2     # 384
VW = NH * DH     # 192
H = 12
B = 2
N_CORES = 8
SCALE = DH ** -0.5

BF = mybir.dt.bfloat16
F32 = mybir.dt.float32

_PROG_CACHE: dict[int, "bacc.Bacc"] = {}


def _build_program(NT: int) -> "bacc.Bacc":
    """Build+schedule+compile the per-core Bass program (SPMD: same program
    on all 8 cores, per-core data differs)."""
    NCH = NT // P      # 128-row chunks (16)
    N5 = NT // 512     # 512-col groups (4)
    NAR = NT // 256    # arena m-blocks per half (8)

    nc = bacc.Bacc(
        "TRN2",
        target_bir_lowering=False,
        debug=False,
        num_devices=N_CORES,
    )

    xT_d = nc.dram_tensor("xT", [C, NT], BF, kind="ExternalInput").ap()
    srcT_d = nc.dram_tensor("srcT", [C, NT], BF, kind="ExternalInput").ap()
    qkw_d = nc.dram_tensor("qk_wT", [C, QW], BF, kind="ExternalInput").ap()
    qksw_d = nc.dram_tensor("qks_wT", [C, QW], BF, kind="ExternalInput").ap()
    vw_d = nc.dram_tensor("v_wT", [C, VW], BF, kind="ExternalInput").ap()
    vsw_d = nc.dram_tensor("vs_wT", [C, VW], BF, kind="ExternalInput").ap()
    pjw_d = nc.dram_tensor("projT", [VW, C], BF, kind="ExternalInput").ap()
    pjsw_d = nc.dram_tensor("projsT", [VW, C], BF, kind="ExternalInput").ap()
    ident_d = nc.dram_tensor("ident", [P, P], BF, kind="ExternalInput").ap()
    oy_d = nc.dram_tensor("out_y", [C, NT], BF, kind="ExternalOutput").ap()
    oys_d = nc.dram_tensor("out_ys", [C, NT], BF, kind="ExternalOutput").ap()

    with tile.TileContext(nc) as tc, ExitStack() as ctx:
        sb = ctx.enter_context(tc.tile_pool(name="sb", bufs=1, space="SBUF"))
        ps = ctx.enter_context(tc.tile_pool(name="ps", bufs=1, space="PSUM"))

        def pt_sim(cols, name):
            return ps.tile([P, cols], F32, tag="ps_sim", bufs=3, name=name,
                           padded_shape=[P, 512])

        def pt_a1(cols, name, rows=P):
            return ps.tile([rows, cols], F32, tag="ps_acc1", bufs=1,
                           name=name, padded_shape=[P, 1024])

        def pt_a2(cols, name, rows=P):
            return ps.tile([rows, cols], F32, tag="ps_acc2", bufs=1,
                           name=name, padded_shape=[P, 1024])

        def pt_small(cols, name):
            return ps.tile([P, cols], F32, tag="ps_small", bufs=1, name=name,
                           padded_shape=[P, 512])

        # ---------------- constants / zero-fills (no DRAM deps)
        ones64 = sb.tile([P, DH], BF, tag="ones64")
        nc.gpsimd.memset(ones64[:], 1.0)
        zb = sb.tile([P, 1], F32, tag="zb")
        nc.gpsimd.memset(zb[:], 0.0)
        ident = sb.tile([P, P], BF, tag="ident")
        nc.sync.dma_start(ident[:], ident_d[:])

        # v tensors, ones-augmented: va[:, u*NCH*65 + 65*i + e] with
        # e in [0,64) = v values, e == 64 = 1.0 (softmax denominator row).
        vax = sb.tile([P, NH * NCH * 65], BF, tag="vax", name="vax")
        vas = sb.tile([P, NH * NCH * 65], BF, tag="vas", name="vas")
        vaxr = vax.rearrange("p (u i e) -> p u i e", u=NH, e=65)
        vasr = vas.rearrange("p (u i e) -> p u i e", u=NH, e=65)
        nc.gpsimd.memset(vaxr[:, :, :, DH:65], 1.0)
        nc.gpsimd.memset(vasr[:, :, :, DH:65], 1.0)

        # ---------------- input loads (emission order == DMA priority)
        xt, qkw, vw = [], [], []
        for i in range(CB):
            qt = sb.tile([P, QW], BF, tag="qkw", bufs=CB, name=f"qkw{i}")
            nc.sync.dma_start(qt[:], qkw_d[P * i:P * (i + 1), :])
            qkw.append(qt)
            vt = sb.tile([P, VW], BF, tag="vw", bufs=CB, name=f"vw{i}")
            nc.sync.dma_start(vt[:], vw_d[P * i:P * (i + 1), :])
            vw.append(vt)
            t = sb.tile([P, NT], BF, tag="xt", bufs=CB, name=f"xt{i}")
            nc.sync.dma_start(t[:], xT_d[P * i:P * (i + 1), :])
            xt.append(t)
        st, qksw = [], []
        for i in range(CB):
            qt = sb.tile([P, QW], BF, tag="qksw", bufs=CB, name=f"qksw{i}")
            nc.sync.dma_start(qt[:], qksw_d[P * i:P * (i + 1), :])
            qksw.append(qt)
            t = sb.tile([P, NT], BF, tag="st", bufs=CB, name=f"st{i}")
            nc.sync.dma_start(t[:], srcT_d[P * i:P * (i + 1), :])
            st.append(t)
        vsw = []
        for i in range(CB):
            vt = sb.tile([P, VW], BF, tag="vsw", bufs=CB, name=f"vsw{i}")
            nc.sync.dma_start(vt[:], vsw_d[P * i:P * (i + 1), :])
            vsw.append(vt)
        # output projection weights; W2 halves padded to K=128 with zeros.
        pjs1 = sb.tile([P, C], BF, tag="pjs1")
        nc.sync.dma_start(pjs1[:], pjsw_d[0:P, :])
        pjs2 = sb.tile([P, C], BF, tag="pjs2")
        nc.sync.dma_start(pjs2[0:DH, :], pjsw_d[P:VW, :])
        nc.gpsimd.memset(pjs2[DH:P, :], 0.0)
        pj1 = sb.tile([P, C], BF, tag="pj1")
        nc.sync.dma_start(pj1[:], pjw_d[0:P, :])
        pj2 = sb.tile([P, C], BF, tag="pj2")
        nc.sync.dma_start(pj2[0:DH, :], pjw_d[P:VW, :])
        nc.gpsimd.memset(pj2[DH:P, :], 0.0)

        # ---------------- per-head transposed QK projections (per 512 group)
        def gen_qkt_group(act_tiles, w_tiles, dst, u, q, pt):
            pst = pt(512, f"psq_u{u}_{q}")
            for cb in range(CB):
                nc.tensor.matmul(
                    pst[:],
                    lhsT=w_tiles[cb][:, D2 * u:D2 * (u + 1)],
                    rhs=act_tiles[cb][:, 512 * q:512 * (q + 1)],
                    start=(cb == 0), stop=(cb == CB - 1),
                )
            nc.vector.tensor_copy(dst[:, 512 * q:512 * (q + 1)], pst[:])

        def mk_qkt_filler(act_tiles, w_tiles, dst, u, qs_):
            def f():
                for q in qs_:
                    gen_qkt_group(act_tiles, w_tiles, dst, u, q, pt_small)
            return f

        # v in natural layout into the pre-built augmented tiles.
        def gen_vaug(act_tiles, w_tiles, vr, tag, chunks, pt=pt_small):
            for i in chunks:
                psv = pt(VW, f"psv_{tag}{i}")
                for cb in range(CB):
                    nc.tensor.matmul(
                        psv[:],
                        lhsT=act_tiles[cb][:, P * i:P * (i + 1)],
                        rhs=w_tiles[cb][:],
                        start=(cb == 0), stop=(cb == CB - 1),
                    )
                nc.vector.tensor_copy(
                    vr[:, :, i, 0:DH],
                    psv.rearrange("p (u e) -> p u e", e=DH),
                )

        # ---------------- merged attention pass (both directions, one head)
        # Software-pipelined by 2 iterations; `fillers` maps iteration index
        # (0..31, h*16+i) -> list of closures emitted at that slot.
        def head_pass(u, qx, qs, fillers, arenas_out, tail=False):
            fillers = dict(fillers or {})
            accS1 = sb.tile([65, NT], BF, tag="accS1", bufs=2,
                            name=f"accS1_{u}")
            pend = []

            def drain(n):
                while len(pend) > n:
                    pend.pop(0)()

            it = 0
            for h in range(2):
                arena_h = sb.tile([P, NAR, NT], BF, tag="arena", bufs=2,
                                  name=f"arena_{u}{h}")
                arenas_out.append(arena_h)
                acc1 = pt_a1(1024, f"acc1_{u}{h}", rows=65)
                for i in range(NCH):
                    est = sb.tile([P, 1024], BF, tag="es", bufs=6,
                                  name=f"es_{u}_{h}_{i}")
                    for q2 in range(2):
                        pst = pt_sim(512, f"ps_{u}{h}{i}{q2}")
                        lo = 1024 * h + 512 * q2
                        nc.tensor.matmul(
                            pst[:],
                            lhsT=qx[:, P * i:P * (i + 1)],
                            rhs=qs[:, lo:lo + 512],
                            start=True, stop=True,
                        )
                        nc.scalar.activation(
                            est[:, 512 * q2:512 * (q2 + 1)], pst[:],
                            mybir.ActivationFunctionType.Exp, bias=zb[:],
                        )
                    # transposes stay on the SP ring: loads are done before
                    # the first transpose and stores alternate to the ACT
                    # ring, so SP is effectively dedicated.  Putting them on
                    # the ACT ring stalls exp behind ring-credit waits.
                    nc.sync.dma_start_transpose(
                        arena_h[:, :, P * i:P * (i + 1)], est[:])

                    def mk_acc(acc1=acc1, est=est, i=i):
                        def f():
                            vsl = vax[:, u * NCH * 65 + 65 * i:
                                      u * NCH * 65 + 65 * (i + 1)]
                            for q2 in range(2):
                                nc.tensor.matmul(
                                    acc1[:, 512 * q2:512 * (q2 + 1)],
                                    lhsT=vsl,
                                    rhs=est[:, 512 * q2:512 * (q2 + 1)],
                                    start=(i == 0), stop=(i == NCH - 1),
                                )
                        return f
                    pend.append(mk_acc())
                    drain(3)
                    for f in fillers.pop(it, []):
                        f()
                    it += 1

                def mk_close(acc1=acc1, h=h):
                    def f():
                        if tail and h == 1:
                            nc.scalar.copy(
                                accS1[:, 1024 * h:1024 * h + 512],
                                acc1[:, 0:512])
                            nc.vector.tensor_copy(
                                accS1[:, 1024 * h + 512:1024 * (h + 1)],
                                acc1[:, 512:1024])
                        else:
                            nc.vector.tensor_copy(
                                accS1[:, 1024 * h:1024 * (h + 1)], acc1[:])
                    return f
                pend.append(mk_close())
            drain(0)
            for fl in fillers.values():   # flush leftovers (shouldn't happen)
                for f in fl:
                    f()
            return accS1

        # dir-2 accumulation closures for (head u, m-half h): one closure per
        # n-half g (16 matmuls each). h==0 stashes into part2; h==1 adds.
        def mk_dir2(u, h, arena_h, part2, accS2):
            out = []
            for g in range(2):
                def f(g=g):
                    acc2 = pt_a2(1024, f"acc2_{u}{h}{g}", rows=65)
                    for j in range(NAR):
                        vsl = vas[:, u * NCH * 65 + 65 * (NAR * h + j):
                                  u * NCH * 65 + 65 * (NAR * h + j + 1)]
                        for k2 in range(2):
                            lo = 1024 * g + 512 * k2
                            nc.tensor.matmul(
                                acc2[:, 512 * k2:512 * (k2 + 1)],
                                lhsT=vsl,
                                rhs=arena_h[:, j, lo:lo + 512],
                                start=(j == 0), stop=(j == NAR - 1),
                            )
                    if h == 0:
                        nc.vector.tensor_copy(
                            part2[:, 1024 * g:1024 * (g + 1)], acc2[:])
                    else:
                        nc.vector.tensor_add(
                            accS2[:, 1024 * g:1024 * (g + 1)], acc2[:],
                            part2[:, 1024 * g:1024 * (g + 1)])
                out.append(f)
            return out

        # ---------------- normalization (psc / bb / mul), baseline machinery
        def make_norm(accS, pname, tail=False):
            state = {}

            def norm_psc():
                psc = pt_small(512, f"psc_{pname}")
                for j in range(NCH):
                    nc.tensor.matmul(
                        psc[:, j:j + 1],
                        lhsT=accS[64:65, P * j:P * (j + 1)],
                        rhs=ones64[64:65, 0:1],
                        start=True, stop=True,
                    )
                rcpF = sb.tile([P, NCH], F32, tag="rcpF", bufs=2,
                               name=f"rcpF_{pname}")
                nc.vector.reciprocal(rcpF[:], psc[:, 0:NCH])
                rcpT = sb.tile([P, NCH], BF, tag="rcpT", bufs=2,
                               name=f"rcpT_{pname}")
                nc.vector.tensor_copy(rcpT[:], rcpF[:])
                state["rcpT"] = rcpT

            def norm_bb():
                rcpT = state["rcpT"]
                bb = sb.tile([DH, NT], BF, tag="bb", bufs=2,
                             name=f"bb_{pname}")
                for k in range(N5):
                    pso = pt_small(512, f"psbb_{pname}_{k}")
                    for jj in range(4):
                        j = 4 * k + jj
                        col = rcpT[:, j:j + 1]
                        lhsT_b = bass.AP(col.tensor, col.offset,
                                         [col.ap[0], [0, DH]])
                        nc.tensor.matmul(
                            pso[0:DH, P * jj:P * (jj + 1)], lhsT=lhsT_b,
                            rhs=ident[:], start=True, stop=True,
                        )
                    if tail:  # ACT is free at the tail: split copy
                        nc.scalar.copy(bb[:, 512 * k:512 * k + 256],
                                       pso[0:DH, 0:256])
                        nc.vector.tensor_copy(
                            bb[:, 512 * k + 256:512 * (k + 1)],
                            pso[0:DH, 256:512])
                    else:
                        nc.vector.tensor_copy(bb[:, 512 * k:512 * (k + 1)],
                                              pso[0:DH, :])
                state["bb"] = bb

            def norm_mul(dst, ks=None):
                bb = state["bb"]
                if ks is None:
                    nc.vector.tensor_mul(dst, accS[0:DH, :], bb[:])
                else:
                    for k in ks:
                        sl = slice(512 * k, 512 * (k + 1))
                        nc.vector.tensor_mul(dst[:, sl], accS[0:DH, sl],
                                             bb[:, sl])
            return norm_psc, norm_bb, norm_mul

        # ---------------- partial output projection, single (cc, k) unit
        def oproj_unit(S1, S2, W1, W2, out_d, pname, cc, k, pt, split=False,
                       dge=None):
            pso = pt(512, f"pso_{pname}_{cc}_{k}")
            nc.tensor.matmul(
                pso[:], lhsT=W1[:, P * cc:P * (cc + 1)],
                rhs=S1[:, 512 * k:512 * (k + 1)],
                start=True, stop=False,
            )
            nc.tensor.matmul(
                pso[:], lhsT=W2[:, P * cc:P * (cc + 1)],
                rhs=S2[:, 512 * k:512 * (k + 1)],
                start=False, stop=True,
            )
            stg = sb.tile([P, 512], BF, tag="stg", bufs=6,
                          name=f"stg_{pname}_{cc}_{k}")
            if split:
                nc.scalar.copy(stg[:, 0:256], pso[:, 0:256])
                nc.vector.tensor_copy(stg[:, 256:512], pso[:, 256:512])
            else:
                nc.vector.tensor_copy(stg[:], pso[:])
            (dge or nc.sync).dma_start(
                out_d[P * cc:P * (cc + 1), 512 * k:512 * (k + 1)],
                stg[:],
            )

        # ---------------- lead-in: head-0 QK projections + first vax chunks
        # qkt tag rotation: qkt0 -> slot0, qkt1 -> slot1; qkt2 gets its own
        # tag (its gen is emitted while qkt0's sim reads are still pending,
        # so slot reuse would stall the gen copies).  qkst: qkst0 -> slot0,
        # qkst1 -> slot1, qkst2 -> slot0 (gen emitted in head-1, after all
        # qkst0 reads).
        qkt = [None] * NH
        qkst = [None] * NH
        qkt[0] = sb.tile([P, NT], BF, tag="qkt", bufs=2, name="qkt0")
        qkst[0] = sb.tile([P, NT], BF, tag="qkst", bufs=2, name="qkst0")

        for q in range(N5):
            gen_qkt_group(xt, qkw, qkt[0], 0, q, pt_sim)
        gen_vaug(xt, vw, vaxr, "vax", range(0, 4), pt_sim)
        for q in range(N5):
            gen_qkt_group(st, qksw, qkst[0], 0, q, pt_sim)

        # per-head dir-2 state
        part2 = [None] * NH
        accS2 = [None] * NH

        def alloc_dir2_state(u):
            part2[u] = sb.tile([65, NT], BF, tag="part2", bufs=1,
                               name=f"part2_{u}")
            accS2[u] = sb.tile([65, NT], BF, tag="accS2", bufs=1,
                               name=f"accS2_{u}")

        # dir2 closures reference arena lists that head_pass fills in as it
        # emits (arena h0 exists by the time h1's fillers run).
        def d2f(arena_list, hidx, u, g):
            def f():
                mk_dir2(u, hidx, arena_list[hidx], part2[u], accS2[u])[g]()
            return f

        # ================ head 0 ================
        alloc_dir2_state(0)
        qkt[1] = sb.tile([P, NT], BF, tag="qkt", bufs=2, name="qkt1")
        qkst[1] = sb.tile([P, NT], BF, tag="qkst", bufs=2, name="qkst1")
        qkt[2] = sb.tile([P, NT], BF, tag="qkt2x", bufs=1, name="qkt2")
        arenas0 = []
        f_vax = [
            (lambda c0: lambda: gen_vaug(xt, vw, vaxr, "vax",
                                         range(c0, c0 + 3)))(c0)
            for c0 in (4, 7, 10, 13)
        ]
        f_vas = [
            (lambda c0: lambda: gen_vaug(st, vsw, vasr, "vas",
                                         range(c0, c0 + 4)))(c0)
            for c0 in (0, 4, 8, 12)
        ]
        fills0 = {
            0: [f_vax[0]], 3: [f_vax[1]], 6: [f_vax[2]], 9: [f_vax[3]],
            11: [f_vas[0]], 14: [f_vas[1]],
            # ---- h1 (x-side gen must finish before Y stacks reuse xt slots)
            16: [f_vas[2]],
            17: [d2f(arenas0, 0, 0, 0)],
            19: [mk_qkt_filler(xt, qkw, qkt[1], 1, (0, 1))],
            21: [f_vas[3]],
            22: [d2f(arenas0, 0, 0, 1)],
            24: [mk_qkt_filler(xt, qkw, qkt[1], 1, (2, 3))],
            26: [mk_qkt_filler(st, qksw, qkst[1], 1, (0, 1))],
            28: [mk_qkt_filler(st, qksw, qkst[1], 1, (2, 3))],
            30: [mk_qkt_filler(xt, qkw, qkt[2], 2, (0, 1))],
            31: [mk_qkt_filler(xt, qkw, qkt[2], 2, (2, 3))],
        }
        accS1_0 = head_pass(0, qkt[0], qkst[0], fills0, arenas0)

        # Y / YS output stacks: reuse the xt tag slots (x-side gen done).
        Y01 = sb.tile([P, NT], BF, tag="xt", bufs=CB, name="Y01")
        Y2 = sb.tile([P, NT], BF, tag="xt", bufs=CB, name="Y2")
        YS01 = sb.tile([P, NT], BF, tag="xt", bufs=CB, name="YS01")
        YS2 = sb.tile([P, NT], BF, tag="xt", bufs=CB, name="YS2")
        nc.gpsimd.memset(Y2[DH:P, :], 0.0)
        nc.gpsimd.memset(YS2[DH:P, :], 0.0)

        def stack_slice(s01, s2, u):
            if u == 0:
                return s01[0:DH, :]
            if u == 1:
                return s01[DH:P, :]
            return s2[0:DH, :]

        # ================ head 1 ================
        alloc_dir2_state(1)
        qkst[2] = sb.tile([P, NT], BF, tag="qkst", bufs=2, name="qkst2")
        n1_0 = make_norm(accS1_0, "n1_0")
        n2_0 = make_norm(accS2[0], "n2_0")
        arenas1 = []
        fills1 = {
            0: [n1_0[0]],
            2: [d2f(arenas0, 1, 0, 0)],
            4: [n1_0[1]],
            6: [lambda: n1_0[2](stack_slice(YS01, YS2, 0))],
            8: [d2f(arenas0, 1, 0, 1)],
            10: [n2_0[0]],
            11: [mk_qkt_filler(st, qksw, qkst[2], 2, (0, 1))],
            13: [n2_0[1]],
            15: [lambda: n2_0[2](stack_slice(Y01, Y2, 0))],
            18: [d2f(arenas1, 0, 1, 0)],
            21: [mk_qkt_filler(st, qksw, qkst[2], 2, (2, 3))],
            24: [d2f(arenas1, 0, 1, 1)],
        }
        accS1_1 = head_pass(1, qkt[1], qkst[1], fills1, arenas1)

        # ================ head 2 ================
        alloc_dir2_state(2)
        n1_1 = make_norm(accS1_1, "n1_1")
        n2_1 = make_norm(accS2[1], "n2_1")
        arenas2 = []
        fills2 = {
            0: [n1_1[0]],
            2: [d2f(arenas1, 1, 1, 0)],
            4: [n1_1[1]],
            6: [lambda: n1_1[2](stack_slice(YS01, YS2, 1))],
            8: [d2f(arenas1, 1, 1, 1)],
            10: [n2_1[0]],
            13: [n2_1[1]],
            15: [lambda: n2_1[2](stack_slice(Y01, Y2, 1))],
            18: [d2f(arenas2, 0, 2, 0)],
            24: [d2f(arenas2, 0, 2, 1)],
        }
        accS1_2 = head_pass(2, qkt[2], qkst[2], fills2, arenas2,
                            tail=True)

        # ================ tail ================
        # norm1(2) -> YS complete -> YS oproj, woven with dir2(2,h1) and
        # norm2(2) -> Y complete -> Y oproj.
        n1_2 = make_norm(accS1_2, "n1_2", tail=True)
        n2_2 = make_norm(accS2[2], "n2_2", tail=True)

        n1_2[0]()
        d2f(arenas2, 1, 2, 0)()
        n1_2[1]()
        n1_2[2](stack_slice(YS01, YS2, 2), ks=range(N5))
        d2f(arenas2, 1, 2, 1)()

        units = [(cc, k) for k in range(N5) for cc in range(CB)]
        # YS oproj units woven around the norm2 chain; rotate 3 psum slots
        # deep (ps_sim slots are idle after the last sim matmul) and
        # alternate store-DMA rings so neither dispatcher backs up.
        ys_rot = [pt_a1, pt_sim, pt_small]
        for idx, (cc, k) in enumerate(units):
            oproj_unit(YS01, YS2, pjs1, pjs2, oys_d, "osrc", cc, k,
                       ys_rot[idx % 3], split=True,
                       dge=nc.sync if idx % 2 == 0 else nc.scalar)
            if idx == 5:
                n2_2[0]()
            elif idx == 11:
                n2_2[1]()
            elif idx == 17:
                n2_2[2](stack_slice(Y01, Y2, 2), ks=range(N5))
        y_rot = [pt_a2, pt_sim, pt_a1, pt_small]
        for idx, (cc, k) in enumerate(units):
            oproj_unit(Y01, Y2, pj1, pj2, oy_d, "oy", cc, k,
                       y_rot[idx % 4], split=True,
                       dge=nc.sync if idx % 2 == 0 else nc.scalar)

    nc.compile()
    return nc


def _get_program(NT: int) -> "bacc.Bacc":
    if NT not in _PROG_CACHE:
        _PROG_CACHE[NT] = _build_program(NT)
    return _PROG_CACHE[NT]


def make_in_maps(x, src, qk_w, qk_src_w, v_w, v_src_w, proj_w, proj_src_w):
    """Host-side sharding: per-core input dicts (pure data marshalling)."""
    bf = ml_dtypes.bfloat16

    def prep(a):
        return np.ascontiguousarray(a).astype(bf)

    in_maps = []
    for c in range(N_CORES):
        b = c // 4
        heads = [3 * (c % 4) + j for j in range(NH)]
        qk_rows = np.concatenate([qk_w[D2 * h:D2 * (h + 1), :] for h in heads])
        qks_rows = np.concatenate(
            [qk_src_w[D2 * h:D2 * (h + 1), :] for h in heads])
        v_rows = np.concatenate([v_w[DH * h:DH * (h + 1), :] for h in heads])
        vs_rows = np.concatenate(
            [v_src_w[DH * h:DH * (h + 1), :] for h in heads])
        pj_cols = np.concatenate(
            [proj_w[:, DH * h:DH * (h + 1)] for h in heads], axis=1)
        pjs_cols = np.concatenate(
            [proj_src_w[:, DH * h:DH * (h + 1)] for h in heads], axis=1)
        in_maps.append({
            "ident": np.eye(P).astype(ml_dtypes.bfloat16),
            "xT": prep(x[b].T),
            "srcT": prep(src[b].T),
            "qk_wT": prep(qk_rows.T * SCALE),
            "qks_wT": prep(qks_rows.T),
            "v_wT": prep(v_rows.T),
            "vs_wT": prep(vs_rows.T),
            "projT": prep(pj_cols.T),
            "projsT": prep(pjs_cols.T),
        })
    return in_maps


LAST_RESULTS = None  # BassKernelResults of the most recent kernel() call
_HOOK_DONE = False


def _install_ntff_hook():
    """The agent image's antenv lacks axon_hooks; inject a stub module and
    register the ctypes NTFF profile hook so trace=True yields exec times."""
    global _HOOK_DONE
    if _HOOK_DONE:
        return
    try:
        import types
        import antenv  # noqa: F401
        if "antenv.axon_hooks" not in sys.modules:
            mod = types.ModuleType("antenv.axon_hooks")
            _hook = [None]
            mod.set_axon_ntff_profile_hook = lambda h: _hook.__setitem__(0, h)
            mod.get_axon_ntff_profile_hook = lambda: _hook[0]
            sys.modules["antenv.axon_hooks"] = mod
        import trn_agent_boot.trn_boot as _tb
        from antenv.axon_hooks import set_axon_ntff_profile_hook
        set_axon_ntff_profile_hook(
            _tb._ntff_profile_via_ctypes("/opt/axon/libaxon_pjrt.so"))
        _HOOK_DONE = True
    except Exception as e:  # profiling is best-effort
        print(f"ntff hook install failed: {e}", file=sys.stderr)


def kernel(x, src, qk_w, qk_src_w, v_w, v_src_w, proj_w, proj_b,
           proj_src_w, proj_src_b):
    global LAST_RESULTS
    x = np.asarray(x, np.float32)
    src = np.asarray(src, np.float32)
    NT = x.shape[1]

    in_maps = make_in_maps(
        x, src,
        np.asarray(qk_w, np.float32), np.asarray(qk_src_w, np.float32),
        np.asarray(v_w, np.float32), np.asarray(v_src_w, np.float32),
        np.asarray(proj_w, np.float32), np.asarray(proj_src_w, np.float32),
    )

    nc = _get_program(NT)
    trace = bool(int(os.environ.get("BCA_TRACE", "0")))
    if trace:
        _install_ntff_hook()
    res = bass_utils.run_bass_kernel_spmd(
        nc, in_maps, core_ids=list(range(N_CORES)), trace=trace,
    )
    LAST_RESULTS = res

    # host gather: sum partial projections over the 4 cores of each batch,
    # transpose back, add biases, concat branches.
    oy = np.zeros((B, NT, C), np.float32)
    oys = np.zeros((B, NT, C), np.float32)
    for c in range(N_CORES):
        b = c // 4
        oy[b] += np.asarray(res.results[c]["out_y"], np.float32).T
        oys[b] += np.asarray(res.results[c]["out_ys"], np.float32).T
    oy += np.asarray(proj_b, np.float32)
    oys += np.asarray(proj_src_b, np.float32)
    return np.concatenate([oy, oys], axis=-1).astype(np.float32)
